# revision 46
# baseline (speedup 1.0000x reference)
"""Trainium2 Bass kernel for single-head attention.

Reference computation (per batch b):
    q = x @ Wq; k = x @ Wk; v = x @ Wv          # x: [S, D], W: [D, D]
    out = softmax(q @ k.T / sqrt(D)) @ v

Shapes: B=4, S=2048, D=1024, f32.

Sharding over 8 NeuronCores: core c -> (batch b = c//2, seq half h = c%2).
Each core:
  - computes q^T, k^T (layout [e, s]) and v ([s, e]) for its own S/2 rows
  - AllGathers k^T (bf16 hi + fp8 lo residual) and v (bf16) within the
    pair {2b, 2b+1}
  - computes scores for its 1024 queries vs all 2048 keys, softmax,
    attn @ v, writes its [1024, 1024] output shard.

dtype strategy (validated empirically):
  - all matmuls in float32r (~13-bit mantissa; end-to-end rel err ~9e-3
    vs the f32 reference, under the 2e-2 gate)
  - attn weights / gathered v in bf16 (error enters output linearly).

Scheduling (v15, ~253us; v6 was ~269us, session baseline 301us):
  - own k^T/v stay resident in SBUF after projection; only the PEER
    half is loaded from the gather output, via a dynamic-offset DMA
    (row index = 1 - partition_id%2).
  - input loads ride the sync queue in priority order ({x s0..3 + wk
    interleaved}, wk4..7, x4..7, wv, wq); load DMA bandwidth is
    ring-shared ~230GB/s, so multi-queue splits don't help. Staging
    rides scalar; collective triggers ride gpsimd; an inline-tensor
    t=0 barrier absorbs the CC engine's ~30us first-collective arming.
  - projections run with [P,512] half-tile PSUM accumulators, k-proj
    c-half-OUTER so matmuls start once {x s0..3, wk} (6.3MB) are in.
  - the 6MB pair exchange is compressed to 5MB in three <=2MB
    ALGO_MESH AllGathers (>2MB falls into ~5x-slower ALGO_RING): kT as
    bf16 hi + fp8e4m3 residual (reconstructed hi+lo on gpsimd, which
    idles during the passes), v as bf16 bitcast into f32r rows.
  - attention is FLASH-STYLE TWO-PASS over the key halves: pass 1
    (own keys: scores -> partial softmax with own max m1/l1 ->
    transpose -> attn@v into an unnormalized bf16 partial) needs no
    peer data and starts right after q-proj (~115us), filling the
    window where a single-pass kernel stalls waiting for the peer's
    k^T (~147us, pair-launch skew ~48us). Pass 2 (peer keys) starts
    ~40us after the gather lands and combines exactly:
      m = max(m1,m2); a = exp((m1-m)/32)
      out = (O1*a + O2) / (l1*a + l2)
    Both passes run the proven PE pipeline with scores two tiles
    ahead of attn@v.
  - pass1's 2-tile drain is folded into pass2's warmup (scores2
    between av1(6)/av1(7)), and av2(6) borrows the then-dead "S" psum
    tag, so the O-psum (bufs=1) never serializes back-to-back av's.
Run-to-run variance: under sustained load the chip drops to the P0
power state (PE 2.38 -> 2.0 GHz); identical NEFFs then measure ~1.2x
slower. 512-col matmul min dur in the trace tells the state: 215ns =
full clock, 256ns = P0. Launch skew between pair cores (~8-20us)
shifts the whole CC chain; the ~10us CC slack absorbs it.
"""

import numpy as np

import concourse.bass as bass
import concourse.mybir as mybir
import concourse.tile as tile
from concourse import bacc
from concourse.bass_utils import run_bass_kernel_spmd

P = 128          # partitions
D = 1024         # model dim (= E)
S_OWN = 1024     # sequence rows per core
S_FULL = 2048
B, NCORES = 4, 8
DT = D // P      # 8 d-tiles
ST = S_OWN // P  # 8 s-tiles
F32 = mybir.dt.float32
F32R = mybir.dt.float32r
BF16 = mybir.dt.bfloat16
FP8 = mybir.dt.float8e4
REPLICA_GROUPS = [[0, 1], [2, 3], [4, 5], [6, 7]]


def build_kernel():
    nc = bacc.Bacc("TRN2", target_bir_lowering=False, num_devices=NCORES)

    x_d = nc.dram_tensor("x", [S_OWN, D], F32, kind="ExternalInput")
    wq_d = nc.dram_tensor("Wq", [D, D], F32, kind="ExternalInput")
    wk_d = nc.dram_tensor("Wk", [D, D], F32, kind="ExternalInput")
    wv_d = nc.dram_tensor("Wv", [D, D], F32, kind="ExternalInput")
    out_d = nc.dram_tensor("out", [S_OWN, D], F32, kind="ExternalOutput")

    # Collective bounce buffers (internal DRAM). Anything over 2MB switches
    # NRT from ALGO_MESH to the ~4-5x slower ALGO_RING (measured: 4MB kT
    # gather 109us, 3MB chunks 81-86us, vs ~25-40us for <=2MB mesh ops),
    # and each mesh op costs ~6us fixed + ~2us gap on the serialized CC
    # engine. So the 6MB exchange is compressed to 5MB in three mesh ops:
    # kT as bf16 hi (2MB) + fp8e4m3 residual lo (1MB) -- numerically ~free,
    # emulated rel err 0.0042 vs 0.0029 for full f32r -- plus v (2MB bf16).
    # All gathers are bitcast into f32r row-tensors.
    send_hi = nc.dram_tensor("send_hi", [4 * P, S_OWN], F32R)
    allc_hi = nc.dram_tensor("allc_hi", [2, 4 * P, S_OWN], F32R)
    send_lo = nc.dram_tensor("send_lo", [2 * P, S_OWN], F32R)
    allc_lo = nc.dram_tensor("allc_lo", [2, 2 * P, S_OWN], F32R)
    send_c = nc.dram_tensor("send_c", [4 * P, S_OWN], F32R)
    allc_c = nc.dram_tensor("allc_c", [2, 4 * P, S_OWN], F32R)

    # bar_send is an inline (NEFF-preloaded) tensor so the t=0 barrier
    # collective has NO producer dependency and triggers immediately; its
    # ~34us CC arming then completes by ~45us instead of ~65us, pulling the
    # whole serialized CC chain (bar, kt0, kt1, v0, v1) ~20us earlier.
    bar_send = nc.inline_tensor(np.zeros((1, 128), np.float32),
                                name="bar_send")
    bar_out = nc.dram_tensor("bar_out", [2, 128], F32)

    ident_np = np.eye(P, dtype=np.float32)
    ident_d = nc.inline_tensor(ident_np, name="ident")

    with tile.TileContext(nc) as tc:
        _emit(nc, tc, x_d, wq_d, wk_d, wv_d, out_d,
              send_hi, allc_hi, send_lo, allc_lo, send_c, allc_c,
              ident_d, bar_send, bar_out)
    nc.compile()
    return nc


def _emit(nc, tc, x_d, wq_d, wk_d, wv_d, out_d,
          send_hi, allc_hi, send_lo, allc_lo, send_c, allc_c,
          ident_d, bar_send, bar_out):
    with tc.tile_pool(name="sb", bufs=1) as sb:
        ident = sb.tile([P, P], F32, name="ident")
        nc.sync.dma_start(ident[:], ident_d.ap())
        identb = sb.tile([P, P], BF16, name="identb")
        nc.gpsimd.dma_start(identb[:], ident_d.ap())  # cast f32->bf16

        # tiny AllGather at t=0: pays the CC engine's ~35-40us
        # first-collective arming latency during the load phase, so the
        # kT gather processes immediately when its data is staged
        nc.gpsimd.collective_compute(
            "AllGather", mybir.AluOpType.bypass,
            replica_groups=REPLICA_GROUPS,
            ins=[bar_send.ap().opt()],
            outs=[bar_out.ap().opt()],
        )

        # which gather-output row is the peer's (0 or 1)
        peer = 1 - (nc.sync.partition_id() % 2)

        # SBUF tag plan (KB/partition, 207.9 usable). Generational reuse:
        #   wk0: wk(8x4K)  -> qT(8x4K)        [wk dies at kT-proj end]
        #   wv0: wv(8x4K)  -> kT_peer(8x4K)   [wv dies at v-proj end]
        #   wq0: wq(8x4K)  -> v_peer(8x2K)    [wq dies at q-proj end]
        #   xT0: xT(8x4K)                     [dies at q-proj end]
        #   kTo: own k^T, 8x4K dedicated
        #   vo:  own v, 8x2K dedicated
        #   xa:  x_nat(3 bufs) -> attn(3 bufs); attnT 2 bufs; stage 2 bufs
        wk_sb = [sb.tile([P, D], F32R, name=f"wk{d}", tag="wk0", bufs=8)
                 for d in range(DT)]
        wv_sb = [sb.tile([P, D], F32R, name=f"wv{d}", tag="wv0", bufs=8)
                 for d in range(DT)]
        wq_sb = [sb.tile([P, D], F32R, name=f"wq{d}", tag="wq0", bufs=8)
                 for d in range(DT)]
        xT = [sb.tile([P, S_OWN], F32R, name=f"xT{d}", tag="xT0", bufs=8)
              for d in range(DT)]
        kT_own = [sb.tile([P, S_OWN], F32R, name=f"kTo{e}", tag="kTo",
                          bufs=8) for e in range(DT)]
        v_own = [sb.tile([P, D], BF16, name=f"vo{s}", tag="vo", bufs=8)
                 for s in range(ST)]

        with tc.tile_pool(name="ps1", bufs=1, space="PSUM") as ps1:
            # ---- input loads: one queue, priority order x/wk, wv, wq.
            # (Measured: load DMA bandwidth is ring-shared ~230GB/s, so
            # splitting loads across sync+scalar queues buys nothing; keep
            # them all on sync so scalar is free for kT staging.) ----
            # Load order: [x0..x3 + wk0..wk3 interleaved, wk4..7, x4..7,
            # wv, wq]. The k projection runs c-half-OUTER below, and its
            # c=0 half needs exactly {x s0..3, all wk} = the first 6.3MB of
            # this stream, so PE projection work starts at ~37us instead of
            # waiting for all of x+wk (~46us).
            x_nats = []
            for s in range(ST):
                x_nat = sb.tile([P, D], F32, name=f"x_nat{s}", tag="xa",
                                bufs=3)
                x_nats.append(x_nat)
            for i in range(4):
                nc.sync.dma_start(x_nats[i][:], x_d.ap()[i * P:(i + 1) * P, :])
                nc.sync.dma_start(
                    wk_sb[i][:], wk_d.ap()[i * P:(i + 1) * P, :].bitcast(F32R))
            for d in range(4, DT):
                nc.sync.dma_start(
                    wk_sb[d][:], wk_d.ap()[d * P:(d + 1) * P, :].bitcast(F32R))
            for s in range(4, ST):
                nc.sync.dma_start(x_nats[s][:], x_d.ap()[s * P:(s + 1) * P, :])
            for d in range(DT):
                nc.sync.dma_start(
                    wv_sb[d][:], wv_d.ap()[d * P:(d + 1) * P, :].bitcast(F32R))
            for d in range(DT):
                nc.sync.dma_start(
                    wq_sb[d][:], wq_d.ap()[d * P:(d + 1) * P, :].bitcast(F32R))

            # ---- x transposes (PE) as tiles arrive ----
            for s in range(ST):
                x_nat = x_nats[s]
                for d in range(DT):
                    pt = ps1.tile([P, P], F32, name=f"pt{s}_{d}", tag="pt",
                                  bufs=2)
                    nc.tensor.transpose(pt[:], x_nat[:, d * P:(d + 1) * P],
                                        ident[:])
                    nc.vector.tensor_copy(xT[d][:, s * P:(s + 1) * P], pt[:])

            # ---- k^T projection, c-half outer -> SBUF -> DRAM -> gather.
            # All projection PSUM tiles are [P, 512] halves (tag "proj",
            # 1 bank each): the c=0 half of every e runs before any c=1
            # work, so matmuls start as soon as x s0..3 + wk are in. ----
            for c in range(2):
                for e in range(DT):
                    pk = ps1.tile([P, 512], F32, name=f"pk{c}_{e}",
                                  tag="proj", bufs=4)
                    for d in range(DT):
                        nc.tensor.matmul(
                            pk[:],
                            wk_sb[d][:, e * P:(e + 1) * P],
                            xT[d][:, c * 512:(c + 1) * 512],
                            start=(d == 0), stop=(d == DT - 1))
                    nc.vector.tensor_copy(
                        kT_own[e][:, c * 512:(c + 1) * 512], pk[:])
                    if c == 1:
                        # hi/lo split for the exchange: hi = bf16(kT),
                        # lo = fp8e4m3(kT - hi) (no scale; subnormal flush
                        # only loses ~0.0005-logit precision).
                        # on GPSIMD (idle here): the vector queue carries
                        # the kT/pv/qt psum copies and stalls the proj psum
                        # ring if these ride it too.
                        hi = sb.tile([P, D], BF16, name=f"hi{e}", tag="hl",
                                     bufs=2)
                        nc.gpsimd.tensor_copy(hi[:], kT_own[e][:])
                        lo = sb.tile([P, D], FP8, name=f"lo{e}", tag="hlo",
                                     bufs=2)
                        nc.gpsimd.tensor_tensor(lo[:], kT_own[e][:], hi[:],
                                                mybir.AluOpType.subtract)
                        nc.scalar.dma_start(
                            send_hi.ap()[e * 64:(e + 1) * 64, :]
                            .bitcast(BF16), hi[:])
                        nc.scalar.dma_start(
                            send_lo.ap()[e * 32:(e + 1) * 32, :]
                            .bitcast(FP8), lo[:])
                        if e == 7:
                            for snd, alc in ((send_hi, allc_hi),
                                             (send_lo, allc_lo)):
                                nc.gpsimd.collective_compute(
                                    "AllGather", mybir.AluOpType.bypass,
                                    replica_groups=REPLICA_GROUPS,
                                    ins=[snd.ap().opt()],
                                    outs=[alc.ap().opt()],
                                )

            # ---- v projection -> own SBUF (bf16) -> send_c (bitcast to
            # f32r: 128 bf16 rows pack into 64 f32r rows) -> AllGather ----
            for s in range(ST):
                for c in range(2):
                    pv = ps1.tile([P, 512], F32, name=f"pv{c}_{s}",
                                  tag="proj", bufs=4)
                    for d in range(DT):
                        nc.tensor.matmul(
                            pv[:],
                            xT[d][:, s * P:(s + 1) * P],
                            wv_sb[d][:, c * 512:(c + 1) * 512],
                            start=(d == 0), stop=(d == DT - 1))
                    nc.vector.tensor_copy(
                        v_own[s][:, c * 512:(c + 1) * 512], pv[:])
                nc.scalar.dma_start(
                    send_c.ap()[s * 64:(s + 1) * 64, :].bitcast(BF16),
                    v_own[s][:])
            nc.gpsimd.collective_compute(
                "AllGather", mybir.AluOpType.bypass,
                replica_groups=REPLICA_GROUPS,
                ins=[send_c.ap().opt()],
                outs=[allc_c.ap().opt()],
            )

            # ---- q^T projection ----
            qT = []
            for e in range(DT):
                qt = sb.tile([P, S_OWN], F32R, name=f"qT{e}", tag="wk0",
                             bufs=8)
                for c in range(2):
                    pq = ps1.tile([P, 512], F32, name=f"pq{c}_{e}",
                                  tag="proj", bufs=4)
                    for d in range(DT):
                        nc.tensor.matmul(
                            pq[:],
                            wq_sb[d][:, e * P:(e + 1) * P],
                            xT[d][:, c * 512:(c + 1) * 512],
                            start=(d == 0), stop=(d == DT - 1))
                    nc.vector.tensor_copy(
                        qt[:, c * 512:(c + 1) * 512], pq[:])
                qT.append(qt)

        # ---- load only the PEER half of the gathers (dynamic row), and
        # reconstruct kT_peer = hi + lo on GPSIMD (idle during the passes;
        # the vector queue would block pass1's softmax work behind these
        # collective-gated adds) ----
        kT_peer = []
        for e in range(DT):
            hi = sb.tile([P, D], BF16, name=f"hip{e}", tag="hl", bufs=2)
            nc.sync.dma_start(
                hi[:],
                allc_hi.ap()[bass.ds(peer, 1), e * 64:(e + 1) * 64, :]
                .bitcast(BF16))
            lo = sb.tile([P, D], FP8, name=f"lop{e}", tag="hlo", bufs=2)
            nc.sync.dma_start(
                lo[:],
                allc_lo.ap()[bass.ds(peer, 1), e * 32:(e + 1) * 32, :]
                .bitcast(FP8))
            t = sb.tile([P, S_OWN], F32R, name=f"kTp{e}", tag="wv0",
                        bufs=8)
            nc.gpsimd.tensor_tensor(t[:], hi[:], lo[:],
                                    mybir.AluOpType.add)
            kT_peer.append(t)
        v_peer = []
        for s in range(ST):
            t = sb.tile([P, D], BF16, name=f"vp{s}", tag="wq0",
                        bufs=8)
            v_peer.append(t)
            nc.sync.dma_start(
                t[:],
                allc_c.ap()[bass.ds(peer, 1),
                            s * 64:(s + 1) * 64, :].bitcast(BF16))

        # ---- attention: flash-style two passes over the key halves.
        # Pass 1 (OWN keys) needs no peer data at all, so it starts
        # right after q-proj (~115us) and fills the window where the
        # baseline stalled waiting for the peer's k^T (~147us). Pass 2
        # (peer keys) starts ~40us after the gather lands -- huge skew
        # margin. Standard flash rescaling makes the result exact:
        #   pass1: m1, l1, O1 = softmax-partial over own keys
        #   pass2: m = max(m1,m2); a = exp((m1-m)/32)
        #          out = (O1*a + O2) / (l1*a + l2)
        kT_half = [kT_own, kT_peer]
        v_half = [v_own, v_peer]
        m1s, l1s, o1s = {}, {}, {}

        with tc.tile_pool(name="ps2", bufs=1, space="PSUM") as ps2:
            state = {}

            def emit_scores(p, sq):
                S_ps = ps2.tile([P, S_OWN], F32, name=f"S{p}_{sq}", tag="S",
                                bufs=2)
                for e in range(DT):
                    for c in range(2):
                        nc.tensor.matmul(
                            S_ps[:, c * 512:(c + 1) * 512],
                            qT[e][:, sq * P:(sq + 1) * P],
                            kT_half[p][e][:, c * 512:(c + 1) * 512],
                            start=(e == 0), stop=(e == DT - 1))
                state[(p, sq)] = S_ps

            def emit_sm1(sq):
                S_ps = state.pop((0, sq))
                m1 = sb.tile([P, 1], F32, name=f"m1_{sq}", tag="m1", bufs=8)
                nc.vector.reduce_max(m1[:], S_ps[:],
                                     axis=mybir.AxisListType.X)
                negm = sb.tile([P, 1], F32, name=f"negm1_{sq}", tag="negm",
                               bufs=2)
                nc.scalar.mul(negm[:], m1[:], -1.0 / 32.0)
                attn = sb.tile([P, S_OWN], BF16, name=f"attn1_{sq}",
                               tag="xa", bufs=3)
                l1 = sb.tile([P, 1], F32, name=f"l1_{sq}", tag="l1", bufs=8)
                nc.scalar.activation(
                    attn[:], S_ps[:], mybir.ActivationFunctionType.Exp,
                    bias=negm[:, 0:1], scale=1.0 / 32.0, accum_out=l1[:])
                m1s[sq], l1s[sq] = m1, l1
                state[(0, sq, "a")] = attn

            def emit_sm2(sq):
                S_ps = state.pop((1, sq))
                m2 = sb.tile([P, 1], F32, name=f"m2_{sq}", tag="m2", bufs=2)
                nc.vector.reduce_max(m2[:], S_ps[:],
                                     axis=mybir.AxisListType.X)
                mm = sb.tile([P, 1], F32, name=f"mm_{sq}", tag="mm", bufs=2)
                nc.vector.tensor_tensor(mm[:], m1s[sq][:], m2[:],
                                        mybir.AluOpType.max)
                negm = sb.tile([P, 1], F32, name=f"negm2_{sq}", tag="negm",
                               bufs=2)
                nc.scalar.mul(negm[:], mm[:], -1.0 / 32.0)
                attn = sb.tile([P, S_OWN], BF16, name=f"attn2_{sq}",
                               tag="xa", bufs=3)
                l2 = sb.tile([P, 1], F32, name=f"l2_{sq}", tag="l2", bufs=2)
                nc.scalar.activation(
                    attn[:], S_ps[:], mybir.ActivationFunctionType.Exp,
                    bias=negm[:, 0:1], scale=1.0 / 32.0, accum_out=l2[:])
                # a = exp((m1 - m)/32); l = l1*a + l2; rl = 1/l
                d1 = sb.tile([P, 1], F32, name=f"d1_{sq}", tag="d1", bufs=2)
                nc.vector.tensor_tensor(d1[:], m1s[sq][:], mm[:],
                                        mybir.AluOpType.subtract)
                alpha = sb.tile([P, 1], F32, name=f"al_{sq}", tag="al",
                                bufs=4)
                nc.scalar.activation(alpha[:], d1[:],
                                     mybir.ActivationFunctionType.Exp,
                                     scale=1.0 / 32.0)
                la = sb.tile([P, 1], F32, name=f"la_{sq}", tag="la", bufs=2)
                nc.vector.tensor_tensor(la[:], l1s[sq][:], alpha[:],
                                        mybir.AluOpType.mult)
                lt = sb.tile([P, 1], F32, name=f"lt_{sq}", tag="lt", bufs=2)
                nc.vector.tensor_tensor(lt[:], la[:], l2[:],
                                        mybir.AluOpType.add)
                rl = sb.tile([P, 1], F32, name=f"rl_{sq}", tag="rl", bufs=4)
                nc.vector.reciprocal(rl[:], lt[:])
                state[(1, sq, "a")] = attn
                state[(sq, "fin")] = (alpha, rl)

            def emit_transp(p, sq):
                # PE block transposes (the XBAR DMA transpose measured only
                # ~45GB/s and collapses under collective DMA contention).
                # All 8 blocks go into ONE psum tile, then ONE vector copy:
                # back-to-back transposes pipeline on the PE instead of
                # round-tripping through per-block copy dependencies.
                attn = state.pop((p, sq, "a"))
                attnT = sb.tile([P, S_OWN], BF16, name=f"aT{p}_{sq}",
                                tag="attnT", bufs=2)
                patb = ps2.tile([P, S_OWN], BF16, name=f"pat{p}_{sq}",
                                tag="pat", bufs=1)
                for t in range(ST):
                    nc.tensor.transpose(
                        patb[:, t * P:(t + 1) * P],
                        attn[:, t * P:(t + 1) * P], identb[:])
                nc.vector.tensor_copy(attnT[:], patb[:])
                state[(p, sq, "T")] = attnT

            def emit_av1(sq):
                attnT = state.pop((0, sq, "T"))
                O_ps = ps2.tile([P, D], F32, name=f"O1_{sq}", tag="O",
                                bufs=1)
                for s in range(ST):
                    for c in range(2):
                        nc.tensor.matmul(
                            O_ps[:, c * 512:(c + 1) * 512],
                            attnT[:, s * P:(s + 1) * P],
                            v_own[s][:, c * 512:(c + 1) * 512],
                            start=(s == 0), stop=(s == ST - 1))
                o1 = sb.tile([P, D], BF16, name=f"o1_{sq}", tag="xT0",
                             bufs=8)
                nc.vector.tensor_copy(o1[:], O_ps[:])
                o1s[sq] = o1

            def emit_av2(sq):
                attnT = state.pop((1, sq, "T"))
                alpha, rl = state.pop((sq, "fin"))
                # sq=6 borrows the (now dead) "S" psum tag so the two drain
                # av2's don't serialize on the single "O" buffer's WAR.
                O_ps = ps2.tile([P, D], F32, name=f"O2_{sq}",
                                tag="S" if sq == 6 else "O",
                                bufs=2 if sq == 6 else 1)
                for s in range(ST):
                    for c in range(2):
                        nc.tensor.matmul(
                            O_ps[:, c * 512:(c + 1) * 512],
                            attnT[:, s * P:(s + 1) * P],
                            v_peer[s][:, c * 512:(c + 1) * 512],
                            start=(s == 0), stop=(s == ST - 1))
                # out = (o1*alpha + O2) * rl, in [P,512] halves: shorter
                # tail after the final matmul, and the halved o_stage tag
                # frees 4KB/partition of SBUF.
                for h in range(2):
                    hs = slice(h * 512, (h + 1) * 512)
                    o_stage = sb.tile([P, 512], F32, name=f"ost{sq}_{h}",
                                      tag="stage", bufs=2)
                    nc.vector.scalar_tensor_tensor(
                        o_stage[:], o1s[sq][:, hs], alpha[:, 0:1],
                        O_ps[:, hs],
                        op0=mybir.AluOpType.mult, op1=mybir.AluOpType.add)
                    nc.vector.tensor_scalar_mul(o_stage[:], o_stage[:],
                                                rl[:, 0:1])
                    nc.scalar.dma_start(
                        out_d.ap()[sq * P:(sq + 1) * P, hs], o_stage[:])

            # pass1 without its drain; pass2 with pass1's last two av's
            # folded into its warmup, so av1(6)/av1(7) have pass2 scores
            # between them instead of stalling back-to-back on the O-psum
            # copy (O bufs=1).
            for sq in range(ST):
                emit_scores(0, sq)
                if sq >= 2:
                    emit_av1(sq - 2)
                emit_sm1(sq)
                emit_transp(0, sq)
            for sq in range(ST + 2):
                if sq < ST:
                    emit_scores(1, sq)
                if sq < 2:
                    emit_av1(ST - 2 + sq)
                else:
                    emit_av2(sq - 2)
                if sq < ST:
                    emit_sm2(sq)
                    emit_transp(1, sq)


_NC_CACHE = {}


def _get_nc():
    if "nc" not in _NC_CACHE:
        _NC_CACHE["nc"] = build_kernel()
    return _NC_CACHE["nc"]


def kernel(x, Wq, Wk, Wv, **_ignored):
    x = np.ascontiguousarray(np.asarray(x, dtype=np.float32))
    Wq = np.ascontiguousarray(np.asarray(Wq, dtype=np.float32))
    Wk = np.ascontiguousarray(np.asarray(Wk, dtype=np.float32))
    Wv = np.ascontiguousarray(np.asarray(Wv, dtype=np.float32))
    nc = _get_nc()
    in_maps = []
    for c in range(NCORES):
        b, h = divmod(c, 2)
        in_maps.append({
            "x": x[b, h * S_OWN:(h + 1) * S_OWN, :],
            "Wq": Wq, "Wk": Wk, "Wv": Wv,
        })
    res = run_bass_kernel_spmd(nc, in_maps, core_ids=list(range(NCORES)))
    out = np.empty((B, S_FULL, D), dtype=np.float32)
    for c in range(NCORES):
        b, h = divmod(c, 2)
        out[b, h * S_OWN:(h + 1) * S_OWN, :] = res.results[c]["out"]
    return out



# revision 47
# speedup vs baseline: 1.0881x; 1.0881x over previous
"""Trainium2 Bass kernel for single-head attention.

Reference computation (per batch b):
    q = x @ Wq; k = x @ Wk; v = x @ Wv          # x: [S, D], W: [D, D]
    out = softmax(q @ k.T / sqrt(D)) @ v

Shapes: B=4, S=2048, D=1024, f32.

Sharding over 8 NeuronCores: core c -> (batch b = c//2, seq half h = c%2).
Each core:
  - computes q^T, k^T (layout [e, s]) and v ([s, e]) for its own S/2 rows
  - AllGathers k^T (bf16 hi + fp8 lo residual) and v (bf16) within the
    pair {2b, 2b+1}
  - computes scores for its 1024 queries vs all 2048 keys, softmax,
    attn @ v, writes its [1024, 1024] output shard.

dtype strategy (validated empirically):
  - all matmuls in float32r (~13-bit mantissa; end-to-end rel err ~9e-3
    vs the f32 reference, under the 2e-2 gate)
  - attn weights / gathered v in bf16 (error enters output linearly).

Scheduling (v15, ~253us; v6 was ~269us, session baseline 301us):
  - own k^T/v stay resident in SBUF after projection; only the PEER
    half is loaded from the gather output, via a dynamic-offset DMA
    (row index = 1 - partition_id%2).
  - input loads ride the sync queue in priority order ({x s0..3 + wk
    interleaved}, wk4..7, x4..7, wv, wq); load DMA bandwidth is
    ring-shared ~230GB/s, so multi-queue splits don't help. Staging
    rides scalar; collective triggers ride gpsimd; an inline-tensor
    t=0 barrier absorbs the CC engine's ~30us first-collective arming.
  - projections run with [P,512] half-tile PSUM accumulators, k-proj
    c-half-OUTER so matmuls start once {x s0..3, wk} (6.3MB) are in.
  - the 6MB pair exchange is compressed to 5MB in three <=2MB
    ALGO_MESH AllGathers (>2MB falls into ~5x-slower ALGO_RING): kT as
    bf16 hi + fp8e4m3 residual (reconstructed hi+lo on gpsimd, which
    idles during the passes), v as bf16 bitcast into f32r rows.
  - attention is FLASH-STYLE TWO-PASS over the key halves: pass 1
    (own keys: scores -> partial softmax with own max m1/l1 ->
    transpose -> attn@v into an unnormalized bf16 partial) needs no
    peer data and starts right after q-proj (~115us), filling the
    window where a single-pass kernel stalls waiting for the peer's
    k^T (~147us, pair-launch skew ~48us). Pass 2 (peer keys) starts
    ~40us after the gather lands and combines exactly:
      m = max(m1,m2); a = exp((m1-m)/32)
      out = (O1*a + O2) / (l1*a + l2)
    Both passes run the proven PE pipeline with scores two tiles
    ahead of attn@v.
  - pass1's 2-tile drain is folded into pass2's warmup (scores2
    between av1(6)/av1(7)), and av2(6) borrows the then-dead "S" psum
    tag, so the O-psum (bufs=1) never serializes back-to-back av's.
Run-to-run variance: under sustained load the chip drops to the P0
power state (PE 2.38 -> 2.0 GHz); identical NEFFs then measure ~1.2x
slower. 512-col matmul min dur in the trace tells the state: 215ns =
full clock, 256ns = P0. Launch skew between pair cores (~8-20us)
shifts the whole CC chain; the ~10us CC slack absorbs it.
"""

import numpy as np

import concourse.bass as bass
import concourse.mybir as mybir
import concourse.tile as tile
from concourse import bacc
from concourse.bass_utils import run_bass_kernel_spmd

P = 128          # partitions
D = 1024         # model dim (= E)
S_OWN = 1024     # sequence rows per core
S_FULL = 2048
B, NCORES = 4, 8
DT = D // P      # 8 d-tiles
ST = S_OWN // P  # 8 s-tiles
F32 = mybir.dt.float32
F32R = mybir.dt.float32r
BF16 = mybir.dt.bfloat16
FP8 = mybir.dt.float8e4
REPLICA_GROUPS = [[0, 1], [2, 3], [4, 5], [6, 7]]


def build_kernel():
    nc = bacc.Bacc("TRN2", target_bir_lowering=False, num_devices=NCORES)

    x_d = nc.dram_tensor("x", [S_OWN, D], F32, kind="ExternalInput")
    wq_d = nc.dram_tensor("Wq", [D, D], F32, kind="ExternalInput")
    wk_d = nc.dram_tensor("Wk", [D, D], F32, kind="ExternalInput")
    wv_d = nc.dram_tensor("Wv", [D, D], F32, kind="ExternalInput")
    out_d = nc.dram_tensor("out", [S_OWN, D], F32, kind="ExternalOutput")

    # Collective bounce buffers (internal DRAM). Anything over 2MB switches
    # NRT from ALGO_MESH to the ~4-5x slower ALGO_RING (measured: 4MB kT
    # gather 109us, 3MB chunks 81-86us, vs ~25-40us for <=2MB mesh ops),
    # and each mesh op costs ~6us fixed + ~2us gap on the serialized CC
    # engine. So the 6MB exchange is compressed to 5MB in three mesh ops:
    # kT as bf16 hi (2MB) + fp8e4m3 residual lo (1MB) -- numerically ~free,
    # emulated rel err 0.0042 vs 0.0029 for full f32r -- plus v (2MB bf16).
    # All gathers are bitcast into f32r row-tensors.
    send_hi = nc.dram_tensor("send_hi", [4 * P, S_OWN], F32R)
    allc_hi = nc.dram_tensor("allc_hi", [2, 4 * P, S_OWN], F32R)
    send_lo = nc.dram_tensor("send_lo", [2 * P, S_OWN], F32R)
    allc_lo = nc.dram_tensor("allc_lo", [2, 2 * P, S_OWN], F32R)
    send_c = nc.dram_tensor("send_c", [4 * P, S_OWN], F32R)
    allc_c = nc.dram_tensor("allc_c", [2, 4 * P, S_OWN], F32R)

    # bar_send is an inline (NEFF-preloaded) tensor so the t=0 barrier
    # collective has NO producer dependency and triggers immediately; its
    # ~34us CC arming then completes by ~45us instead of ~65us, pulling the
    # whole serialized CC chain (bar, kt0, kt1, v0, v1) ~20us earlier.
    bar_send = nc.inline_tensor(np.zeros((1, 128), np.float32),
                                name="bar_send")
    bar_out = nc.dram_tensor("bar_out", [2, 128], F32)

    ident_np = np.eye(P, dtype=np.float32)
    ident_d = nc.inline_tensor(ident_np, name="ident")

    with tile.TileContext(nc) as tc:
        _emit(nc, tc, x_d, wq_d, wk_d, wv_d, out_d,
              send_hi, allc_hi, send_lo, allc_lo, send_c, allc_c,
              ident_d, bar_send, bar_out)
    nc.compile()
    return nc


def _emit(nc, tc, x_d, wq_d, wk_d, wv_d, out_d,
          send_hi, allc_hi, send_lo, allc_lo, send_c, allc_c,
          ident_d, bar_send, bar_out):
    with tc.tile_pool(name="sb", bufs=1) as sb:
        ident = sb.tile([P, P], F32, name="ident")
        nc.sync.dma_start(ident[:], ident_d.ap())
        identb = sb.tile([P, P], BF16, name="identb")
        nc.gpsimd.dma_start(identb[:], ident_d.ap())  # cast f32->bf16

        # tiny AllGather at t=0: pays the CC engine's ~35-40us
        # first-collective arming latency during the load phase, so the
        # kT gather processes immediately when its data is staged
        nc.gpsimd.collective_compute(
            "AllGather", mybir.AluOpType.bypass,
            replica_groups=REPLICA_GROUPS,
            ins=[bar_send.ap().opt()],
            outs=[bar_out.ap().opt()],
        )

        # which gather-output row is the peer's (0 or 1)
        peer = 1 - (nc.sync.partition_id() % 2)

        # SBUF tag plan (KB/partition, 207.9 usable). Generational reuse:
        #   wk0: wk(8x4K)  -> qT(8x4K)        [wk dies at kT-proj end]
        #   wv0: wv(8x4K)  -> kT_peer(8x4K)   [wv dies at v-proj end]
        #   wq0: wq(8x4K)  -> v_peer(8x2K)    [wq dies at q-proj end]
        #   xT0: xT(8x4K)                     [dies at q-proj end]
        #   kTo: own k^T, 8x4K dedicated
        #   vo:  own v, 8x2K dedicated
        #   xa:  x_nat(3 bufs) -> attn(3 bufs); attnT 2 bufs; stage 2 bufs
        wk_sb = [sb.tile([P, D], F32R, name=f"wk{d}", tag="wk0", bufs=8)
                 for d in range(DT)]
        wv_sb = [sb.tile([P, D], F32R, name=f"wv{d}", tag="wv0", bufs=8)
                 for d in range(DT)]
        wq_sb = [sb.tile([P, D], F32R, name=f"wq{d}", tag="wq0", bufs=8)
                 for d in range(DT)]
        xT = [sb.tile([P, S_OWN], F32R, name=f"xT{d}", tag="xT0", bufs=8)
              for d in range(DT)]
        kT_own = [sb.tile([P, S_OWN], F32R, name=f"kTo{e}", tag="kTo",
                          bufs=8) for e in range(DT)]
        v_own = [sb.tile([P, D], BF16, name=f"vo{s}", tag="vo", bufs=8)
                 for s in range(ST)]

        with tc.tile_pool(name="ps1", bufs=1, space="PSUM") as ps1:
            # ---- input loads: one queue, priority order x/wk, wv, wq.
            # (Measured: load DMA bandwidth is ring-shared ~230GB/s, so
            # splitting loads across sync+scalar queues buys nothing; keep
            # them all on sync so scalar is free for kT staging.) ----
            # Load order: [x0..x3 + wk0..wk3 interleaved, wk4..7, x4..7,
            # wv, wq]. The k projection runs c-half-OUTER below, and its
            # c=0 half needs exactly {x s0..3, all wk} = the first 6.3MB of
            # this stream, so PE projection work starts at ~37us instead of
            # waiting for all of x+wk (~46us).
            x_nats = []
            for s in range(ST):
                x_nat = sb.tile([P, D], F32, name=f"x_nat{s}", tag="xa",
                                bufs=3)
                x_nats.append(x_nat)
            for i in range(4):
                nc.sync.dma_start(x_nats[i][:], x_d.ap()[i * P:(i + 1) * P, :])
                nc.sync.dma_start(
                    wk_sb[i][:], wk_d.ap()[i * P:(i + 1) * P, :].bitcast(F32R))
            for d in range(4, DT):
                nc.sync.dma_start(
                    wk_sb[d][:], wk_d.ap()[d * P:(d + 1) * P, :].bitcast(F32R))
            for s in range(4, ST):
                nc.sync.dma_start(x_nats[s][:], x_d.ap()[s * P:(s + 1) * P, :])
            for d in range(DT):
                nc.sync.dma_start(
                    wv_sb[d][:], wv_d.ap()[d * P:(d + 1) * P, :].bitcast(F32R))
            for d in range(DT):
                nc.sync.dma_start(
                    wq_sb[d][:], wq_d.ap()[d * P:(d + 1) * P, :].bitcast(F32R))

            # ---- x transposes (PE) as tiles arrive ----
            for s in range(ST):
                x_nat = x_nats[s]
                for d in range(DT):
                    pt = ps1.tile([P, P], F32, name=f"pt{s}_{d}", tag="pt",
                                  bufs=2)
                    nc.tensor.transpose(pt[:], x_nat[:, d * P:(d + 1) * P],
                                        ident[:])
                    nc.vector.tensor_copy(xT[d][:, s * P:(s + 1) * P], pt[:])

            # ---- k^T projection, c-half outer -> SBUF -> DRAM -> gather.
            # All projection PSUM tiles are [P, 512] halves (tag "proj",
            # 1 bank each): the c=0 half of every e runs before any c=1
            # work, so matmuls start as soon as x s0..3 + wk are in. ----
            for c in range(2):
                for e in range(DT):
                    pk = ps1.tile([P, 512], F32, name=f"pk{c}_{e}",
                                  tag="proj", bufs=4)
                    for d in range(DT):
                        nc.tensor.matmul(
                            pk[:],
                            wk_sb[d][:, e * P:(e + 1) * P],
                            xT[d][:, c * 512:(c + 1) * 512],
                            start=(d == 0), stop=(d == DT - 1))
                    nc.vector.tensor_copy(
                        kT_own[e][:, c * 512:(c + 1) * 512], pk[:])
                    if c == 1:
                        # hi/lo split for the exchange: hi = bf16(kT),
                        # lo = fp8e4m3(kT - hi) (no scale; subnormal flush
                        # only loses ~0.0005-logit precision).
                        # on vector: gpsimd's software DVE runs these ~4x
                        # slower (3.5us/cast) and would delay the gather
                        # staging by ~30us.
                        hi = sb.tile([P, D], BF16, name=f"hi{e}", tag="hl",
                                     bufs=2)
                        nc.vector.tensor_copy(hi[:], kT_own[e][:])
                        lo = sb.tile([P, D], FP8, name=f"lo{e}", tag="hlo",
                                     bufs=2)
                        nc.vector.tensor_tensor(lo[:], kT_own[e][:], hi[:],
                                                mybir.AluOpType.subtract)
                        nc.scalar.dma_start(
                            send_hi.ap()[e * 64:(e + 1) * 64, :]
                            .bitcast(BF16), hi[:])
                        nc.scalar.dma_start(
                            send_lo.ap()[e * 32:(e + 1) * 32, :]
                            .bitcast(FP8), lo[:])
                        if e == 7:
                            for snd, alc in ((send_hi, allc_hi),
                                             (send_lo, allc_lo)):
                                nc.gpsimd.collective_compute(
                                    "AllGather", mybir.AluOpType.bypass,
                                    replica_groups=REPLICA_GROUPS,
                                    ins=[snd.ap().opt()],
                                    outs=[alc.ap().opt()],
                                )

            # ---- v projection -> own SBUF (bf16) -> send_c (bitcast to
            # f32r: 128 bf16 rows pack into 64 f32r rows) -> AllGather ----
            for s in range(ST):
                for c in range(2):
                    pv = ps1.tile([P, 512], F32, name=f"pv{c}_{s}",
                                  tag="proj", bufs=4)
                    for d in range(DT):
                        nc.tensor.matmul(
                            pv[:],
                            xT[d][:, s * P:(s + 1) * P],
                            wv_sb[d][:, c * 512:(c + 1) * 512],
                            start=(d == 0), stop=(d == DT - 1))
                    nc.vector.tensor_copy(
                        v_own[s][:, c * 512:(c + 1) * 512], pv[:])
                nc.scalar.dma_start(
                    send_c.ap()[s * 64:(s + 1) * 64, :].bitcast(BF16),
                    v_own[s][:])
            nc.gpsimd.collective_compute(
                "AllGather", mybir.AluOpType.bypass,
                replica_groups=REPLICA_GROUPS,
                ins=[send_c.ap().opt()],
                outs=[allc_c.ap().opt()],
            )

            # ---- q^T projection ----
            qT = []
            for e in range(DT):
                qt = sb.tile([P, S_OWN], F32R, name=f"qT{e}", tag="wk0",
                             bufs=8)
                for c in range(2):
                    pq = ps1.tile([P, 512], F32, name=f"pq{c}_{e}",
                                  tag="proj", bufs=4)
                    for d in range(DT):
                        nc.tensor.matmul(
                            pq[:],
                            wq_sb[d][:, e * P:(e + 1) * P],
                            xT[d][:, c * 512:(c + 1) * 512],
                            start=(d == 0), stop=(d == DT - 1))
                    nc.vector.tensor_copy(
                        qt[:, c * 512:(c + 1) * 512], pq[:])
                qT.append(qt)

        # ---- load only the PEER half of the gathers (dynamic row), and
        # reconstruct kT_peer = hi + lo on GPSIMD (idle during the passes;
        # the vector queue would block pass1's softmax work behind these
        # collective-gated adds) ----
        kT_peer = []
        for e in range(DT):
            hi = sb.tile([P, D], BF16, name=f"hip{e}", tag="hl", bufs=2)
            nc.sync.dma_start(
                hi[:],
                allc_hi.ap()[bass.ds(peer, 1), e * 64:(e + 1) * 64, :]
                .bitcast(BF16))
            lo = sb.tile([P, D], FP8, name=f"lop{e}", tag="hlo", bufs=2)
            nc.sync.dma_start(
                lo[:],
                allc_lo.ap()[bass.ds(peer, 1), e * 32:(e + 1) * 32, :]
                .bitcast(FP8))
            t = sb.tile([P, S_OWN], F32R, name=f"kTp{e}", tag="wv0",
                        bufs=8)
            nc.gpsimd.tensor_tensor(t[:], hi[:], lo[:],
                                    mybir.AluOpType.add)
            kT_peer.append(t)
        v_peer = []
        for s in range(ST):
            t = sb.tile([P, D], BF16, name=f"vp{s}", tag="wq0",
                        bufs=8)
            v_peer.append(t)
            nc.sync.dma_start(
                t[:],
                allc_c.ap()[bass.ds(peer, 1),
                            s * 64:(s + 1) * 64, :].bitcast(BF16))

        # ---- attention: flash-style two passes over the key halves.
        # Pass 1 (OWN keys) needs no peer data at all, so it starts
        # right after q-proj (~115us) and fills the window where the
        # baseline stalled waiting for the peer's k^T (~147us). Pass 2
        # (peer keys) starts ~40us after the gather lands -- huge skew
        # margin. Standard flash rescaling makes the result exact:
        #   pass1: m1, l1, O1 = softmax-partial over own keys
        #   pass2: m = max(m1,m2); a = exp((m1-m)/32)
        #          out = (O1*a + O2) / (l1*a + l2)
        kT_half = [kT_own, kT_peer]
        v_half = [v_own, v_peer]
        m1s, l1s, o1s = {}, {}, {}

        with tc.tile_pool(name="ps2", bufs=1, space="PSUM") as ps2:
            state = {}

            def emit_scores(p, sq):
                S_ps = ps2.tile([P, S_OWN], F32, name=f"S{p}_{sq}", tag="S",
                                bufs=2)
                for e in range(DT):
                    for c in range(2):
                        nc.tensor.matmul(
                            S_ps[:, c * 512:(c + 1) * 512],
                            qT[e][:, sq * P:(sq + 1) * P],
                            kT_half[p][e][:, c * 512:(c + 1) * 512],
                            start=(e == 0), stop=(e == DT - 1))
                state[(p, sq)] = S_ps

            def emit_sm1(sq):
                S_ps = state.pop((0, sq))
                m1 = sb.tile([P, 1], F32, name=f"m1_{sq}", tag="m1", bufs=8)
                nc.vector.reduce_max(m1[:], S_ps[:],
                                     axis=mybir.AxisListType.X)
                negm = sb.tile([P, 1], F32, name=f"negm1_{sq}", tag="negm",
                               bufs=2)
                nc.scalar.mul(negm[:], m1[:], -1.0 / 32.0)
                attn = sb.tile([P, S_OWN], BF16, name=f"attn1_{sq}",
                               tag="xa", bufs=3)
                l1 = sb.tile([P, 1], F32, name=f"l1_{sq}", tag="l1", bufs=8)
                nc.scalar.activation(
                    attn[:], S_ps[:], mybir.ActivationFunctionType.Exp,
                    bias=negm[:, 0:1], scale=1.0 / 32.0, accum_out=l1[:])
                m1s[sq], l1s[sq] = m1, l1
                state[(0, sq, "a")] = attn

            def emit_sm2(sq):
                S_ps = state.pop((1, sq))
                m2 = sb.tile([P, 1], F32, name=f"m2_{sq}", tag="m2", bufs=2)
                nc.vector.reduce_max(m2[:], S_ps[:],
                                     axis=mybir.AxisListType.X)
                mm = sb.tile([P, 1], F32, name=f"mm_{sq}", tag="mm", bufs=2)
                nc.vector.tensor_tensor(mm[:], m1s[sq][:], m2[:],
                                        mybir.AluOpType.max)
                negm = sb.tile([P, 1], F32, name=f"negm2_{sq}", tag="negm",
                               bufs=2)
                nc.scalar.mul(negm[:], mm[:], -1.0 / 32.0)
                attn = sb.tile([P, S_OWN], BF16, name=f"attn2_{sq}",
                               tag="xa", bufs=3)
                l2 = sb.tile([P, 1], F32, name=f"l2_{sq}", tag="l2", bufs=2)
                nc.scalar.activation(
                    attn[:], S_ps[:], mybir.ActivationFunctionType.Exp,
                    bias=negm[:, 0:1], scale=1.0 / 32.0, accum_out=l2[:])
                # a = exp((m1 - m)/32); l = l1*a + l2; rl = 1/l
                d1 = sb.tile([P, 1], F32, name=f"d1_{sq}", tag="d1", bufs=2)
                nc.vector.tensor_tensor(d1[:], m1s[sq][:], mm[:],
                                        mybir.AluOpType.subtract)
                alpha = sb.tile([P, 1], F32, name=f"al_{sq}", tag="al",
                                bufs=4)
                nc.scalar.activation(alpha[:], d1[:],
                                     mybir.ActivationFunctionType.Exp,
                                     scale=1.0 / 32.0)
                la = sb.tile([P, 1], F32, name=f"la_{sq}", tag="la", bufs=2)
                nc.vector.tensor_tensor(la[:], l1s[sq][:], alpha[:],
                                        mybir.AluOpType.mult)
                lt = sb.tile([P, 1], F32, name=f"lt_{sq}", tag="lt", bufs=2)
                nc.vector.tensor_tensor(lt[:], la[:], l2[:],
                                        mybir.AluOpType.add)
                rl = sb.tile([P, 1], F32, name=f"rl_{sq}", tag="rl", bufs=4)
                nc.vector.reciprocal(rl[:], lt[:])
                state[(1, sq, "a")] = attn
                state[(sq, "fin")] = (alpha, rl)

            def emit_transp(p, sq):
                # PE block transposes (the XBAR DMA transpose measured only
                # ~45GB/s and collapses under collective DMA contention).
                # All 8 blocks go into ONE psum tile, then ONE vector copy:
                # back-to-back transposes pipeline on the PE instead of
                # round-tripping through per-block copy dependencies.
                attn = state.pop((p, sq, "a"))
                attnT = sb.tile([P, S_OWN], BF16, name=f"aT{p}_{sq}",
                                tag="attnT", bufs=2)
                patb = ps2.tile([P, S_OWN], BF16, name=f"pat{p}_{sq}",
                                tag="pat", bufs=1)
                for t in range(ST):
                    nc.tensor.transpose(
                        patb[:, t * P:(t + 1) * P],
                        attn[:, t * P:(t + 1) * P], identb[:])
                nc.vector.tensor_copy(attnT[:], patb[:])
                state[(p, sq, "T")] = attnT

            def emit_av1(sq):
                attnT = state.pop((0, sq, "T"))
                O_ps = ps2.tile([P, D], F32, name=f"O1_{sq}", tag="O",
                                bufs=1)
                for s in range(ST):
                    for c in range(2):
                        nc.tensor.matmul(
                            O_ps[:, c * 512:(c + 1) * 512],
                            attnT[:, s * P:(s + 1) * P],
                            v_own[s][:, c * 512:(c + 1) * 512],
                            start=(s == 0), stop=(s == ST - 1))
                o1 = sb.tile([P, D], BF16, name=f"o1_{sq}", tag="xT0",
                             bufs=8)
                nc.vector.tensor_copy(o1[:], O_ps[:])
                o1s[sq] = o1

            def emit_av2(sq):
                attnT = state.pop((1, sq, "T"))
                alpha, rl = state.pop((sq, "fin"))
                # sq=6 borrows the (now dead) "S" psum tag so the two drain
                # av2's don't serialize on the single "O" buffer's WAR.
                O_ps = ps2.tile([P, D], F32, name=f"O2_{sq}",
                                tag="S" if sq == 6 else "O",
                                bufs=2 if sq == 6 else 1)
                for s in range(ST):
                    for c in range(2):
                        nc.tensor.matmul(
                            O_ps[:, c * 512:(c + 1) * 512],
                            attnT[:, s * P:(s + 1) * P],
                            v_peer[s][:, c * 512:(c + 1) * 512],
                            start=(s == 0), stop=(s == ST - 1))
                # out = (o1*alpha + O2) * rl, in [P,512] halves: shorter
                # tail after the final matmul, and the halved o_stage tag
                # frees 4KB/partition of SBUF.
                for h in range(2):
                    hs = slice(h * 512, (h + 1) * 512)
                    o_stage = sb.tile([P, 512], F32, name=f"ost{sq}_{h}",
                                      tag="stage", bufs=2)
                    nc.vector.scalar_tensor_tensor(
                        o_stage[:], o1s[sq][:, hs], alpha[:, 0:1],
                        O_ps[:, hs],
                        op0=mybir.AluOpType.mult, op1=mybir.AluOpType.add)
                    nc.vector.tensor_scalar_mul(o_stage[:], o_stage[:],
                                                rl[:, 0:1])
                    nc.scalar.dma_start(
                        out_d.ap()[sq * P:(sq + 1) * P, hs], o_stage[:])

            # pass1 without its drain; pass2 with pass1's last two av's
            # folded into its warmup, so av1(6)/av1(7) have pass2 scores
            # between them instead of stalling back-to-back on the O-psum
            # copy (O bufs=1).
            for sq in range(ST):
                emit_scores(0, sq)
                if sq >= 2:
                    emit_av1(sq - 2)
                emit_sm1(sq)
                emit_transp(0, sq)
            for sq in range(ST + 2):
                if sq < ST:
                    emit_scores(1, sq)
                if sq < 2:
                    emit_av1(ST - 2 + sq)
                else:
                    emit_av2(sq - 2)
                if sq < ST:
                    emit_sm2(sq)
                    emit_transp(1, sq)


_NC_CACHE = {}


def _get_nc():
    if "nc" not in _NC_CACHE:
        _NC_CACHE["nc"] = build_kernel()
    return _NC_CACHE["nc"]


def kernel(x, Wq, Wk, Wv, **_ignored):
    x = np.ascontiguousarray(np.asarray(x, dtype=np.float32))
    Wq = np.ascontiguousarray(np.asarray(Wq, dtype=np.float32))
    Wk = np.ascontiguousarray(np.asarray(Wk, dtype=np.float32))
    Wv = np.ascontiguousarray(np.asarray(Wv, dtype=np.float32))
    nc = _get_nc()
    in_maps = []
    for c in range(NCORES):
        b, h = divmod(c, 2)
        in_maps.append({
            "x": x[b, h * S_OWN:(h + 1) * S_OWN, :],
            "Wq": Wq, "Wk": Wk, "Wv": Wv,
        })
    res = run_bass_kernel_spmd(nc, in_maps, core_ids=list(range(NCORES)))
    out = np.empty((B, S_FULL, D), dtype=np.float32)
    for c in range(NCORES):
        b, h = divmod(c, 2)
        out[b, h * S_OWN:(h + 1) * S_OWN, :] = res.results[c]["out"]
    return out



# revision 48
# speedup vs baseline: 1.1113x; 1.0212x over previous
"""Trainium2 Bass kernel for single-head attention.

Reference computation (per batch b):
    q = x @ Wq; k = x @ Wk; v = x @ Wv          # x: [S, D], W: [D, D]
    out = softmax(q @ k.T / sqrt(D)) @ v

Shapes: B=4, S=2048, D=1024, f32.

Sharding over 8 NeuronCores: core c -> (batch b = c//2, seq half h = c%2).
Each core:
  - computes q^T, k^T (layout [e, s]) and v ([s, e]) for its own S/2 rows
  - AllGathers k^T (bf16 hi + fp8 lo residual) and v (bf16) within the
    pair {2b, 2b+1}
  - computes scores for its 1024 queries vs all 2048 keys, softmax,
    attn @ v, writes its [1024, 1024] output shard.

dtype strategy (validated empirically):
  - all matmuls in float32r (~13-bit mantissa; end-to-end rel err ~9e-3
    vs the f32 reference, under the 2e-2 gate)
  - attn weights / gathered v in bf16 (error enters output linearly).

Scheduling (v15, ~253us; v6 was ~269us, session baseline 301us):
  - own k^T/v stay resident in SBUF after projection; only the PEER
    half is loaded from the gather output, via a dynamic-offset DMA
    (row index = 1 - partition_id%2).
  - input loads ride the sync queue in priority order ({x s0..3 + wk
    interleaved}, wk4..7, x4..7, wv, wq); load DMA bandwidth is
    ring-shared ~230GB/s, so multi-queue splits don't help. Staging
    rides scalar; collective triggers ride gpsimd; an inline-tensor
    t=0 barrier absorbs the CC engine's ~30us first-collective arming.
  - projections run with [P,512] half-tile PSUM accumulators, k-proj
    c-half-OUTER so matmuls start once {x s0..3, wk} (6.3MB) are in.
  - the 6MB pair exchange is compressed to 5MB in three <=2MB
    ALGO_MESH AllGathers (>2MB falls into ~5x-slower ALGO_RING): kT as
    bf16 hi + fp8e4m3 residual (reconstructed hi+lo on gpsimd, which
    idles during the passes), v as bf16 bitcast into f32r rows.
  - attention is FLASH-STYLE TWO-PASS over the key halves: pass 1
    (own keys: scores -> partial softmax with own max m1/l1 ->
    transpose -> attn@v into an unnormalized bf16 partial) needs no
    peer data and starts right after q-proj (~115us), filling the
    window where a single-pass kernel stalls waiting for the peer's
    k^T (~147us, pair-launch skew ~48us). Pass 2 (peer keys) starts
    ~40us after the gather lands and combines exactly:
      m = max(m1,m2); a = exp((m1-m)/32)
      out = (O1*a + O2) / (l1*a + l2)
    Both passes run the proven PE pipeline with scores two tiles
    ahead of attn@v.
  - pass1's 2-tile drain is folded into pass2's warmup (scores2
    between av1(6)/av1(7)), and av2(6) borrows the then-dead "S" psum
    tag, so the O-psum (bufs=1) never serializes back-to-back av's.
Run-to-run variance: under sustained load the chip drops to the P0
power state (PE 2.38 -> 2.0 GHz); identical NEFFs then measure ~1.2x
slower. 512-col matmul min dur in the trace tells the state: 215ns =
full clock, 256ns = P0. Launch skew between pair cores (~8-20us)
shifts the whole CC chain; the ~10us CC slack absorbs it.
"""

import numpy as np

import concourse.bass as bass
import concourse.mybir as mybir
import concourse.tile as tile
from concourse import bacc
from concourse.bass_utils import run_bass_kernel_spmd

P = 128          # partitions
D = 1024         # model dim (= E)
S_OWN = 1024     # sequence rows per core
S_FULL = 2048
B, NCORES = 4, 8
DT = D // P      # 8 d-tiles
ST = S_OWN // P  # 8 s-tiles
F32 = mybir.dt.float32
F32R = mybir.dt.float32r
BF16 = mybir.dt.bfloat16
FP8 = mybir.dt.float8e4
REPLICA_GROUPS = [[0, 1], [2, 3], [4, 5], [6, 7]]


def build_kernel():
    nc = bacc.Bacc("TRN2", target_bir_lowering=False, num_devices=NCORES)

    x_d = nc.dram_tensor("x", [S_OWN, D], F32, kind="ExternalInput")
    wq_d = nc.dram_tensor("Wq", [D, D], F32, kind="ExternalInput")
    wk_d = nc.dram_tensor("Wk", [D, D], F32, kind="ExternalInput")
    wv_d = nc.dram_tensor("Wv", [D, D], F32, kind="ExternalInput")
    out_d = nc.dram_tensor("out", [S_OWN, D], F32, kind="ExternalOutput")

    # Collective bounce buffers (internal DRAM). Anything over 2MB switches
    # NRT from ALGO_MESH to the ~4-5x slower ALGO_RING (measured: 4MB kT
    # gather 109us, 3MB chunks 81-86us, vs ~25-40us for <=2MB mesh ops),
    # and each mesh op costs ~6us fixed + ~2us gap on the serialized CC
    # engine. So the 6MB exchange is compressed to 5MB in three mesh ops:
    # kT as bf16 hi (2MB) + fp8e4m3 residual lo (1MB) -- numerically ~free,
    # emulated rel err 0.0042 vs 0.0029 for full f32r -- plus v (2MB bf16).
    # All gathers are bitcast into f32r row-tensors.
    send_hi = nc.dram_tensor("send_hi", [4 * P, S_OWN], F32R)
    allc_hi = nc.dram_tensor("allc_hi", [2, 4 * P, S_OWN], F32R)
    send_lo = nc.dram_tensor("send_lo", [2 * P, S_OWN], F32R)
    allc_lo = nc.dram_tensor("allc_lo", [2, 2 * P, S_OWN], F32R)
    send_c = nc.dram_tensor("send_c", [4 * P, S_OWN], F32R)
    allc_c = nc.dram_tensor("allc_c", [2, 4 * P, S_OWN], F32R)

    # bar_send is an inline (NEFF-preloaded) tensor so the t=0 barrier
    # collective has NO producer dependency and triggers immediately; its
    # ~34us CC arming then completes by ~45us instead of ~65us, pulling the
    # whole serialized CC chain (bar, kt0, kt1, v0, v1) ~20us earlier.
    bar_send = nc.inline_tensor(np.zeros((1, 128), np.float32),
                                name="bar_send")
    bar_out = nc.dram_tensor("bar_out", [2, 128], F32)

    ident_np = np.eye(P, dtype=np.float32)
    ident_d = nc.inline_tensor(ident_np, name="ident")

    with tile.TileContext(nc) as tc:
        _emit(nc, tc, x_d, wq_d, wk_d, wv_d, out_d,
              send_hi, allc_hi, send_lo, allc_lo, send_c, allc_c,
              ident_d, bar_send, bar_out)
    nc.compile()
    return nc


def _emit(nc, tc, x_d, wq_d, wk_d, wv_d, out_d,
          send_hi, allc_hi, send_lo, allc_lo, send_c, allc_c,
          ident_d, bar_send, bar_out):
    with tc.tile_pool(name="sb", bufs=1) as sb:
        ident = sb.tile([P, P], F32, name="ident")
        nc.sync.dma_start(ident[:], ident_d.ap())
        identb = sb.tile([P, P], BF16, name="identb")
        nc.gpsimd.dma_start(identb[:], ident_d.ap())  # cast f32->bf16

        # tiny AllGather at t=0: pays the CC engine's ~35-40us
        # first-collective arming latency during the load phase, so the
        # kT gather processes immediately when its data is staged
        nc.gpsimd.collective_compute(
            "AllGather", mybir.AluOpType.bypass,
            replica_groups=REPLICA_GROUPS,
            ins=[bar_send.ap().opt()],
            outs=[bar_out.ap().opt()],
        )

        # which gather-output row is the peer's (0 or 1)
        peer = 1 - (nc.sync.partition_id() % 2)

        # SBUF tag plan (KB/partition, 207.9 usable). Generational reuse:
        #   wk0: wk(8x4K)  -> qT(8x4K)        [wk dies at kT-proj end]
        #   wv0: wv(8x4K)  -> kT_peer(8x4K)   [wv dies at v-proj end]
        #   wq0: wq(8x4K)  -> v_peer(8x2K)    [wq dies at q-proj end]
        #   xT0: xT(8x4K)                     [dies at q-proj end]
        #   kTo: own k^T, 8x4K dedicated
        #   vo:  own v, 8x2K dedicated
        #   xa:  x_nat(3 bufs) -> attn(3 bufs); attnT 2 bufs; stage 2 bufs
        wk_sb = [sb.tile([P, D], F32R, name=f"wk{d}", tag="wk0", bufs=8)
                 for d in range(DT)]
        wv_sb = [sb.tile([P, D], F32R, name=f"wv{d}", tag="wv0", bufs=8)
                 for d in range(DT)]
        wq_sb = [sb.tile([P, D], F32R, name=f"wq{d}", tag="wq0", bufs=8)
                 for d in range(DT)]
        xT = [sb.tile([P, S_OWN], F32R, name=f"xT{d}", tag="xT0", bufs=8)
              for d in range(DT)]
        kT_own = [sb.tile([P, S_OWN], F32R, name=f"kTo{e}", tag="kTo",
                          bufs=8) for e in range(DT)]
        v_own = [sb.tile([P, D], BF16, name=f"vo{s}", tag="vo", bufs=8)
                 for s in range(ST)]

        with tc.tile_pool(name="ps1", bufs=1, space="PSUM") as ps1:
            # ---- input loads: one queue, priority order x/wk, wv, wq.
            # (Measured: load DMA bandwidth is ring-shared ~230GB/s, so
            # splitting loads across sync+scalar queues buys nothing; keep
            # them all on sync so scalar is free for kT staging.) ----
            # Load order: [x0..x3 + wk0..wk3 interleaved, wk4..7, x4..7,
            # wv, wq]. The k projection runs c-half-OUTER below, and its
            # c=0 half needs exactly {x s0..3, all wk} = the first 6.3MB of
            # this stream, so PE projection work starts at ~37us instead of
            # waiting for all of x+wk (~46us).
            x_nats = []
            for s in range(ST):
                x_nat = sb.tile([P, D], F32, name=f"x_nat{s}", tag="xa",
                                bufs=3)
                x_nats.append(x_nat)
            for i in range(4):
                nc.sync.dma_start(x_nats[i][:], x_d.ap()[i * P:(i + 1) * P, :])
                nc.sync.dma_start(
                    wk_sb[i][:], wk_d.ap()[i * P:(i + 1) * P, :].bitcast(F32R))
            for d in range(4, DT):
                nc.sync.dma_start(
                    wk_sb[d][:], wk_d.ap()[d * P:(d + 1) * P, :].bitcast(F32R))
            for s in range(4, ST):
                nc.sync.dma_start(x_nats[s][:], x_d.ap()[s * P:(s + 1) * P, :])
            for d in range(DT):
                nc.sync.dma_start(
                    wv_sb[d][:], wv_d.ap()[d * P:(d + 1) * P, :].bitcast(F32R))
            for d in range(DT):
                nc.sync.dma_start(
                    wq_sb[d][:], wq_d.ap()[d * P:(d + 1) * P, :].bitcast(F32R))

            # ---- x transposes (PE) as tiles arrive ----
            for s in range(ST):
                x_nat = x_nats[s]
                for d in range(DT):
                    pt = ps1.tile([P, P], F32, name=f"pt{s}_{d}", tag="pt",
                                  bufs=2)
                    nc.tensor.transpose(pt[:], x_nat[:, d * P:(d + 1) * P],
                                        ident[:])
                    nc.vector.tensor_copy(xT[d][:, s * P:(s + 1) * P], pt[:])

            # ---- k^T projection, c-half outer -> SBUF -> DRAM -> gather.
            # All projection PSUM tiles are [P, 512] halves (tag "proj",
            # 1 bank each): the c=0 half of every e runs before any c=1
            # work, so matmuls start as soon as x s0..3 + wk are in. ----
            for c in range(2):
                for e in range(DT):
                    pk = ps1.tile([P, 512], F32, name=f"pk{c}_{e}",
                                  tag="proj", bufs=4)
                    for d in range(DT):
                        nc.tensor.matmul(
                            pk[:],
                            wk_sb[d][:, e * P:(e + 1) * P],
                            xT[d][:, c * 512:(c + 1) * 512],
                            start=(d == 0), stop=(d == DT - 1))
                    nc.vector.tensor_copy(
                        kT_own[e][:, c * 512:(c + 1) * 512], pk[:])
                    if c == 1:
                        # hi/lo split for the exchange: hi = bf16(kT),
                        # lo = fp8e4m3(kT - hi) (no scale; subnormal flush
                        # only loses ~0.0005-logit precision).
                        # hi-cast on the idle scalar engine (mul by 1.0);
                        # the subtract needs two inputs so it rides vector.
                        # (gpsimd's software DVE is ~4x slower - 3.5us/cast
                        # - and would delay the gather staging by ~30us.)
                        hi = sb.tile([P, D], BF16, name=f"hi{e}", tag="hl",
                                     bufs=2)
                        nc.scalar.mul(hi[:], kT_own[e][:], 1.0)
                        lo = sb.tile([P, D], FP8, name=f"lo{e}", tag="hlo",
                                     bufs=2)
                        nc.vector.tensor_tensor(lo[:], kT_own[e][:], hi[:],
                                                mybir.AluOpType.subtract)
                        nc.scalar.dma_start(
                            send_hi.ap()[e * 64:(e + 1) * 64, :]
                            .bitcast(BF16), hi[:])
                        nc.scalar.dma_start(
                            send_lo.ap()[e * 32:(e + 1) * 32, :]
                            .bitcast(FP8), lo[:])
                        if e == 7:
                            for snd, alc in ((send_hi, allc_hi),
                                             (send_lo, allc_lo)):
                                nc.gpsimd.collective_compute(
                                    "AllGather", mybir.AluOpType.bypass,
                                    replica_groups=REPLICA_GROUPS,
                                    ins=[snd.ap().opt()],
                                    outs=[alc.ap().opt()],
                                )

            # ---- v projection -> own SBUF (bf16) -> send_c (bitcast to
            # f32r: 128 bf16 rows pack into 64 f32r rows) -> AllGather ----
            for s in range(ST):
                for c in range(2):
                    pv = ps1.tile([P, 512], F32, name=f"pv{c}_{s}",
                                  tag="proj", bufs=4)
                    for d in range(DT):
                        nc.tensor.matmul(
                            pv[:],
                            xT[d][:, s * P:(s + 1) * P],
                            wv_sb[d][:, c * 512:(c + 1) * 512],
                            start=(d == 0), stop=(d == DT - 1))
                    nc.vector.tensor_copy(
                        v_own[s][:, c * 512:(c + 1) * 512], pv[:])
                nc.scalar.dma_start(
                    send_c.ap()[s * 64:(s + 1) * 64, :].bitcast(BF16),
                    v_own[s][:])
            nc.gpsimd.collective_compute(
                "AllGather", mybir.AluOpType.bypass,
                replica_groups=REPLICA_GROUPS,
                ins=[send_c.ap().opt()],
                outs=[allc_c.ap().opt()],
            )

            # ---- q^T projection ----
            qT = []
            for e in range(DT):
                qt = sb.tile([P, S_OWN], F32R, name=f"qT{e}", tag="wk0",
                             bufs=8)
                for c in range(2):
                    pq = ps1.tile([P, 512], F32, name=f"pq{c}_{e}",
                                  tag="proj", bufs=4)
                    for d in range(DT):
                        nc.tensor.matmul(
                            pq[:],
                            wq_sb[d][:, e * P:(e + 1) * P],
                            xT[d][:, c * 512:(c + 1) * 512],
                            start=(d == 0), stop=(d == DT - 1))
                    nc.vector.tensor_copy(
                        qt[:, c * 512:(c + 1) * 512], pq[:])
                qT.append(qt)

        # ---- load only the PEER half of the gathers (dynamic row), and
        # reconstruct kT_peer = hi + lo on GPSIMD (idle during the passes;
        # the vector queue would block pass1's softmax work behind these
        # collective-gated adds) ----
        kT_peer = []
        for e in range(DT):
            hi = sb.tile([P, D], BF16, name=f"hip{e}", tag="hl", bufs=2)
            nc.sync.dma_start(
                hi[:],
                allc_hi.ap()[bass.ds(peer, 1), e * 64:(e + 1) * 64, :]
                .bitcast(BF16))
            lo = sb.tile([P, D], FP8, name=f"lop{e}", tag="hlo", bufs=2)
            nc.sync.dma_start(
                lo[:],
                allc_lo.ap()[bass.ds(peer, 1), e * 32:(e + 1) * 32, :]
                .bitcast(FP8))
            t = sb.tile([P, S_OWN], F32R, name=f"kTp{e}", tag="wv0",
                        bufs=8)
            nc.gpsimd.tensor_tensor(t[:], hi[:], lo[:],
                                    mybir.AluOpType.add)
            kT_peer.append(t)
        v_peer = []
        for s in range(ST):
            t = sb.tile([P, D], BF16, name=f"vp{s}", tag="wq0",
                        bufs=8)
            v_peer.append(t)
            nc.sync.dma_start(
                t[:],
                allc_c.ap()[bass.ds(peer, 1),
                            s * 64:(s + 1) * 64, :].bitcast(BF16))

        # ---- attention: flash-style two passes over the key halves.
        # Pass 1 (OWN keys) needs no peer data at all, so it starts
        # right after q-proj (~115us) and fills the window where the
        # baseline stalled waiting for the peer's k^T (~147us). Pass 2
        # (peer keys) starts ~40us after the gather lands -- huge skew
        # margin. Standard flash rescaling makes the result exact:
        #   pass1: m1, l1, O1 = softmax-partial over own keys
        #   pass2: m = max(m1,m2); a = exp((m1-m)/32)
        #          out = (O1*a + O2) / (l1*a + l2)
        kT_half = [kT_own, kT_peer]
        v_half = [v_own, v_peer]
        m1s, l1s, o1s = {}, {}, {}

        with tc.tile_pool(name="ps2", bufs=1, space="PSUM") as ps2:
            state = {}

            def emit_scores(p, sq):
                S_ps = ps2.tile([P, S_OWN], F32, name=f"S{p}_{sq}", tag="S",
                                bufs=2)
                for e in range(DT):
                    for c in range(2):
                        nc.tensor.matmul(
                            S_ps[:, c * 512:(c + 1) * 512],
                            qT[e][:, sq * P:(sq + 1) * P],
                            kT_half[p][e][:, c * 512:(c + 1) * 512],
                            start=(e == 0), stop=(e == DT - 1))
                state[(p, sq)] = S_ps

            def emit_sm1(sq):
                S_ps = state.pop((0, sq))
                m1 = sb.tile([P, 1], F32, name=f"m1_{sq}", tag="m1", bufs=8)
                nc.vector.reduce_max(m1[:], S_ps[:],
                                     axis=mybir.AxisListType.X)
                negm = sb.tile([P, 1], F32, name=f"negm1_{sq}", tag="negm",
                               bufs=2)
                nc.scalar.mul(negm[:], m1[:], -1.0 / 32.0)
                attn = sb.tile([P, S_OWN], BF16, name=f"attn1_{sq}",
                               tag="xa", bufs=3)
                l1 = sb.tile([P, 1], F32, name=f"l1_{sq}", tag="l1", bufs=8)
                nc.scalar.activation(
                    attn[:], S_ps[:], mybir.ActivationFunctionType.Exp,
                    bias=negm[:, 0:1], scale=1.0 / 32.0, accum_out=l1[:])
                m1s[sq], l1s[sq] = m1, l1
                state[(0, sq, "a")] = attn

            def emit_sm2(sq):
                S_ps = state.pop((1, sq))
                m2 = sb.tile([P, 1], F32, name=f"m2_{sq}", tag="m2", bufs=2)
                nc.vector.reduce_max(m2[:], S_ps[:],
                                     axis=mybir.AxisListType.X)
                mm = sb.tile([P, 1], F32, name=f"mm_{sq}", tag="mm", bufs=2)
                nc.vector.tensor_tensor(mm[:], m1s[sq][:], m2[:],
                                        mybir.AluOpType.max)
                negm = sb.tile([P, 1], F32, name=f"negm2_{sq}", tag="negm",
                               bufs=2)
                nc.scalar.mul(negm[:], mm[:], -1.0 / 32.0)
                attn = sb.tile([P, S_OWN], BF16, name=f"attn2_{sq}",
                               tag="xa", bufs=3)
                l2 = sb.tile([P, 1], F32, name=f"l2_{sq}", tag="l2", bufs=2)
                nc.scalar.activation(
                    attn[:], S_ps[:], mybir.ActivationFunctionType.Exp,
                    bias=negm[:, 0:1], scale=1.0 / 32.0, accum_out=l2[:])
                # a = exp((m1 - m)/32); l = l1*a + l2; rl = 1/l
                d1 = sb.tile([P, 1], F32, name=f"d1_{sq}", tag="d1", bufs=2)
                nc.vector.tensor_tensor(d1[:], m1s[sq][:], mm[:],
                                        mybir.AluOpType.subtract)
                alpha = sb.tile([P, 1], F32, name=f"al_{sq}", tag="al",
                                bufs=4)
                nc.scalar.activation(alpha[:], d1[:],
                                     mybir.ActivationFunctionType.Exp,
                                     scale=1.0 / 32.0)
                la = sb.tile([P, 1], F32, name=f"la_{sq}", tag="la", bufs=2)
                nc.vector.tensor_tensor(la[:], l1s[sq][:], alpha[:],
                                        mybir.AluOpType.mult)
                lt = sb.tile([P, 1], F32, name=f"lt_{sq}", tag="lt", bufs=2)
                nc.vector.tensor_tensor(lt[:], la[:], l2[:],
                                        mybir.AluOpType.add)
                rl = sb.tile([P, 1], F32, name=f"rl_{sq}", tag="rl", bufs=4)
                nc.vector.reciprocal(rl[:], lt[:])
                state[(1, sq, "a")] = attn
                state[(sq, "fin")] = (alpha, rl)

            def emit_transp(p, sq):
                # PE block transposes (the XBAR DMA transpose measured only
                # ~45GB/s and collapses under collective DMA contention).
                # All 8 blocks go into ONE psum tile, then ONE vector copy:
                # back-to-back transposes pipeline on the PE instead of
                # round-tripping through per-block copy dependencies.
                attn = state.pop((p, sq, "a"))
                attnT = sb.tile([P, S_OWN], BF16, name=f"aT{p}_{sq}",
                                tag="attnT", bufs=2)
                patb = ps2.tile([P, S_OWN], BF16, name=f"pat{p}_{sq}",
                                tag="pat", bufs=1)
                for t in range(ST):
                    nc.tensor.transpose(
                        patb[:, t * P:(t + 1) * P],
                        attn[:, t * P:(t + 1) * P], identb[:])
                nc.vector.tensor_copy(attnT[:], patb[:])
                state[(p, sq, "T")] = attnT

            def emit_av1(sq):
                attnT = state.pop((0, sq, "T"))
                O_ps = ps2.tile([P, D], F32, name=f"O1_{sq}", tag="O",
                                bufs=1)
                for s in range(ST):
                    for c in range(2):
                        nc.tensor.matmul(
                            O_ps[:, c * 512:(c + 1) * 512],
                            attnT[:, s * P:(s + 1) * P],
                            v_own[s][:, c * 512:(c + 1) * 512],
                            start=(s == 0), stop=(s == ST - 1))
                o1 = sb.tile([P, D], BF16, name=f"o1_{sq}", tag="xT0",
                             bufs=8)
                nc.vector.tensor_copy(o1[:], O_ps[:])
                o1s[sq] = o1

            def emit_av2(sq):
                attnT = state.pop((1, sq, "T"))
                alpha, rl = state.pop((sq, "fin"))
                # sq=6 borrows the (now dead) "S" psum tag so the two drain
                # av2's don't serialize on the single "O" buffer's WAR.
                O_ps = ps2.tile([P, D], F32, name=f"O2_{sq}",
                                tag="S" if sq == 6 else "O",
                                bufs=2 if sq == 6 else 1)
                for s in range(ST):
                    for c in range(2):
                        nc.tensor.matmul(
                            O_ps[:, c * 512:(c + 1) * 512],
                            attnT[:, s * P:(s + 1) * P],
                            v_peer[s][:, c * 512:(c + 1) * 512],
                            start=(s == 0), stop=(s == ST - 1))
                # out = (o1*alpha + O2) * rl, in [P,512] halves: shorter
                # tail after the final matmul, and the halved o_stage tag
                # frees 4KB/partition of SBUF.
                for h in range(2):
                    hs = slice(h * 512, (h + 1) * 512)
                    o_stage = sb.tile([P, 512], F32, name=f"ost{sq}_{h}",
                                      tag="stage", bufs=2)
                    nc.vector.scalar_tensor_tensor(
                        o_stage[:], o1s[sq][:, hs], alpha[:, 0:1],
                        O_ps[:, hs],
                        op0=mybir.AluOpType.mult, op1=mybir.AluOpType.add)
                    nc.vector.tensor_scalar_mul(o_stage[:], o_stage[:],
                                                rl[:, 0:1])
                    nc.scalar.dma_start(
                        out_d.ap()[sq * P:(sq + 1) * P, hs], o_stage[:])

            # pass1 without its drain; pass2 with pass1's last two av's
            # folded into its warmup, so av1(6)/av1(7) have pass2 scores
            # between them instead of stalling back-to-back on the O-psum
            # copy (O bufs=1).
            for sq in range(ST):
                emit_scores(0, sq)
                if sq >= 2:
                    emit_av1(sq - 2)
                emit_sm1(sq)
                emit_transp(0, sq)
            for sq in range(ST + 2):
                if sq < ST:
                    emit_scores(1, sq)
                if sq < 2:
                    emit_av1(ST - 2 + sq)
                else:
                    emit_av2(sq - 2)
                if sq < ST:
                    emit_sm2(sq)
                    emit_transp(1, sq)


_NC_CACHE = {}


def _get_nc():
    if "nc" not in _NC_CACHE:
        _NC_CACHE["nc"] = build_kernel()
    return _NC_CACHE["nc"]


def kernel(x, Wq, Wk, Wv, **_ignored):
    x = np.ascontiguousarray(np.asarray(x, dtype=np.float32))
    Wq = np.ascontiguousarray(np.asarray(Wq, dtype=np.float32))
    Wk = np.ascontiguousarray(np.asarray(Wk, dtype=np.float32))
    Wv = np.ascontiguousarray(np.asarray(Wv, dtype=np.float32))
    nc = _get_nc()
    in_maps = []
    for c in range(NCORES):
        b, h = divmod(c, 2)
        in_maps.append({
            "x": x[b, h * S_OWN:(h + 1) * S_OWN, :],
            "Wq": Wq, "Wk": Wk, "Wv": Wv,
        })
    res = run_bass_kernel_spmd(nc, in_maps, core_ids=list(range(NCORES)))
    out = np.empty((B, S_FULL, D), dtype=np.float32)
    for c in range(NCORES):
        b, h = divmod(c, 2)
        out[b, h * S_OWN:(h + 1) * S_OWN, :] = res.results[c]["out"]
    return out



# revision 49
# speedup vs baseline: 1.1116x; 1.0003x over previous
"""Trainium2 Bass kernel for single-head attention.

Reference computation (per batch b):
    q = x @ Wq; k = x @ Wk; v = x @ Wv          # x: [S, D], W: [D, D]
    out = softmax(q @ k.T / sqrt(D)) @ v

Shapes: B=4, S=2048, D=1024, f32.

Sharding over 8 NeuronCores: core c -> (batch b = c//2, seq half h = c%2).
Each core:
  - computes q^T, k^T (layout [e, s]) and v ([s, e]) for its own S/2 rows
  - AllGathers k^T (bf16 hi + fp8 lo residual) and v (bf16) within the
    pair {2b, 2b+1}
  - computes scores for its 1024 queries vs all 2048 keys, softmax,
    attn @ v, writes its [1024, 1024] output shard.

dtype strategy (validated empirically):
  - all matmuls in float32r (~13-bit mantissa; end-to-end rel err ~9e-3
    vs the f32 reference, under the 2e-2 gate)
  - attn weights / gathered v in bf16 (error enters output linearly).

Scheduling (v15, ~253us; v6 was ~269us, session baseline 301us):
  - own k^T/v stay resident in SBUF after projection; only the PEER
    half is loaded from the gather output, via a dynamic-offset DMA
    (row index = 1 - partition_id%2).
  - input loads ride the sync queue in priority order ({x s0..3 + wk
    interleaved}, wk4..7, x4..7, wv, wq); load DMA bandwidth is
    ring-shared ~230GB/s, so multi-queue splits don't help. Staging
    rides scalar; collective triggers ride gpsimd; an inline-tensor
    t=0 barrier absorbs the CC engine's ~30us first-collective arming.
  - projections run with [P,512] half-tile PSUM accumulators, k-proj
    c-half-OUTER so matmuls start once {x s0..3, wk} (6.3MB) are in.
  - the 6MB pair exchange is compressed to 5MB in three <=2MB
    ALGO_MESH AllGathers (>2MB falls into ~5x-slower ALGO_RING): kT as
    bf16 hi + fp8e4m3 residual (reconstructed hi+lo on gpsimd, which
    idles during the passes), v as bf16 bitcast into f32r rows.
  - attention is FLASH-STYLE TWO-PASS over the key halves: pass 1
    (own keys: scores -> partial softmax with own max m1/l1 ->
    transpose -> attn@v into an unnormalized bf16 partial) needs no
    peer data and starts right after q-proj (~115us), filling the
    window where a single-pass kernel stalls waiting for the peer's
    k^T (~147us, pair-launch skew ~48us). Pass 2 (peer keys) starts
    ~40us after the gather lands and combines exactly:
      m = max(m1,m2); a = exp((m1-m)/32)
      out = (O1*a + O2) / (l1*a + l2)
    Both passes run the proven PE pipeline with scores two tiles
    ahead of attn@v.
  - pass1's 2-tile drain is folded into pass2's warmup (scores2
    between av1(6)/av1(7)), and av2(6) borrows the then-dead "S" psum
    tag, so the O-psum (bufs=1) never serializes back-to-back av's.
Run-to-run variance: under sustained load the chip drops to the P0
power state (PE 2.38 -> 2.0 GHz); identical NEFFs then measure ~1.2x
slower. 512-col matmul min dur in the trace tells the state: 215ns =
full clock, 256ns = P0. Launch skew between pair cores (~8-20us)
shifts the whole CC chain; the ~10us CC slack absorbs it.
"""

import numpy as np

import concourse.bass as bass
import concourse.mybir as mybir
import concourse.tile as tile
from concourse import bacc
from concourse.bass_utils import run_bass_kernel_spmd

P = 128          # partitions
D = 1024         # model dim (= E)
S_OWN = 1024     # sequence rows per core
S_FULL = 2048
B, NCORES = 4, 8
DT = D // P      # 8 d-tiles
ST = S_OWN // P  # 8 s-tiles
F32 = mybir.dt.float32
F32R = mybir.dt.float32r
BF16 = mybir.dt.bfloat16
FP8 = mybir.dt.float8e4
REPLICA_GROUPS = [[0, 1], [2, 3], [4, 5], [6, 7]]


def build_kernel():
    nc = bacc.Bacc("TRN2", target_bir_lowering=False, num_devices=NCORES)

    x_d = nc.dram_tensor("x", [S_OWN, D], F32, kind="ExternalInput")
    wq_d = nc.dram_tensor("Wq", [D, D], F32, kind="ExternalInput")
    wk_d = nc.dram_tensor("Wk", [D, D], F32, kind="ExternalInput")
    wv_d = nc.dram_tensor("Wv", [D, D], F32, kind="ExternalInput")
    out_d = nc.dram_tensor("out", [S_OWN, D], F32, kind="ExternalOutput")

    # Collective bounce buffers (internal DRAM). Anything over 2MB switches
    # NRT from ALGO_MESH to the ~4-5x slower ALGO_RING (measured: 4MB kT
    # gather 109us, 3MB chunks 81-86us, vs ~25-40us for <=2MB mesh ops),
    # and each mesh op costs ~6us fixed + ~2us gap on the serialized CC
    # engine. So the 6MB exchange is compressed to 5MB in three mesh ops:
    # kT as bf16 hi (2MB) + fp8e4m3 residual lo (1MB) -- numerically ~free,
    # emulated rel err 0.0042 vs 0.0029 for full f32r -- plus v (2MB bf16).
    # All gathers are bitcast into f32r row-tensors.
    send_hi = nc.dram_tensor("send_hi", [4 * P, S_OWN], F32R)
    allc_hi = nc.dram_tensor("allc_hi", [2, 4 * P, S_OWN], F32R)
    send_lo = nc.dram_tensor("send_lo", [2 * P, S_OWN], F32R)
    allc_lo = nc.dram_tensor("allc_lo", [2, 2 * P, S_OWN], F32R)
    send_c = nc.dram_tensor("send_c", [4 * P, S_OWN], F32R)
    allc_c = nc.dram_tensor("allc_c", [2, 4 * P, S_OWN], F32R)

    # bar_send is an inline (NEFF-preloaded) tensor so the t=0 barrier
    # collective has NO producer dependency and triggers immediately; its
    # ~34us CC arming then completes by ~45us instead of ~65us, pulling the
    # whole serialized CC chain (bar, kt0, kt1, v0, v1) ~20us earlier.
    bar_send = nc.inline_tensor(np.zeros((1, 128), np.float32),
                                name="bar_send")
    bar_out = nc.dram_tensor("bar_out", [2, 128], F32)

    ident_np = np.eye(P, dtype=np.float32)
    ident_d = nc.inline_tensor(ident_np, name="ident")

    with tile.TileContext(nc) as tc:
        _emit(nc, tc, x_d, wq_d, wk_d, wv_d, out_d,
              send_hi, allc_hi, send_lo, allc_lo, send_c, allc_c,
              ident_d, bar_send, bar_out)
    nc.compile()
    return nc


def _emit(nc, tc, x_d, wq_d, wk_d, wv_d, out_d,
          send_hi, allc_hi, send_lo, allc_lo, send_c, allc_c,
          ident_d, bar_send, bar_out):
    with tc.tile_pool(name="sb", bufs=1) as sb:
        ident = sb.tile([P, P], F32, name="ident")
        nc.sync.dma_start(ident[:], ident_d.ap())
        identb = sb.tile([P, P], BF16, name="identb")
        nc.gpsimd.dma_start(identb[:], ident_d.ap())  # cast f32->bf16

        # tiny AllGather at t=0: pays the CC engine's ~35-40us
        # first-collective arming latency during the load phase, so the
        # kT gather processes immediately when its data is staged
        nc.gpsimd.collective_compute(
            "AllGather", mybir.AluOpType.bypass,
            replica_groups=REPLICA_GROUPS,
            ins=[bar_send.ap().opt()],
            outs=[bar_out.ap().opt()],
        )

        # which gather-output row is the peer's (0 or 1)
        peer = 1 - (nc.sync.partition_id() % 2)

        # SBUF tag plan (KB/partition, 207.9 usable). Generational reuse:
        #   wk0: wk(8x4K)  -> qT(8x4K)        [wk dies at kT-proj end]
        #   wv0: wv(8x4K)  -> kT_peer(8x4K)   [wv dies at v-proj end]
        #   wq0: wq(8x4K)  -> v_peer(8x2K)    [wq dies at q-proj end]
        #   xT0: xT(8x4K)                     [dies at q-proj end]
        #   kTo: own k^T, 8x4K dedicated
        #   vo:  own v, 8x2K dedicated
        #   xa:  x_nat(3 bufs) -> attn(3 bufs); attnT 2 bufs; stage 2 bufs
        wk_sb = [sb.tile([P, D], F32R, name=f"wk{d}", tag="wk0", bufs=8)
                 for d in range(DT)]
        wv_sb = [sb.tile([P, D], F32R, name=f"wv{d}", tag="wv0", bufs=8)
                 for d in range(DT)]
        wq_sb = [sb.tile([P, D], F32R, name=f"wq{d}", tag="wq0", bufs=8)
                 for d in range(DT)]
        xT = [sb.tile([P, S_OWN], F32R, name=f"xT{d}", tag="xT0", bufs=8)
              for d in range(DT)]
        kT_own = [sb.tile([P, S_OWN], F32R, name=f"kTo{e}", tag="kTo",
                          bufs=8) for e in range(DT)]
        v_own = [sb.tile([P, D], BF16, name=f"vo{s}", tag="vo", bufs=8)
                 for s in range(ST)]

        with tc.tile_pool(name="ps1", bufs=1, space="PSUM") as ps1:
            # ---- input loads: one queue, priority order x/wk, wv, wq.
            # (Measured: load DMA bandwidth is ring-shared ~230GB/s, so
            # splitting loads across sync+scalar queues buys nothing; keep
            # them all on sync so scalar is free for kT staging.) ----
            # Load order: [x0..x3 + wk0..wk3 interleaved, wk4..7, x4..7,
            # wv, wq]. The k projection runs c-half-OUTER below, and its
            # c=0 half needs exactly {x s0..3, all wk} = the first 6.3MB of
            # this stream, so PE projection work starts at ~37us instead of
            # waiting for all of x+wk (~46us).
            x_nats = []
            for s in range(ST):
                x_nat = sb.tile([P, D], F32, name=f"x_nat{s}", tag="xa",
                                bufs=3)
                x_nats.append(x_nat)
            for i in range(4):
                nc.sync.dma_start(x_nats[i][:], x_d.ap()[i * P:(i + 1) * P, :])
                nc.sync.dma_start(
                    wk_sb[i][:], wk_d.ap()[i * P:(i + 1) * P, :].bitcast(F32R))
            for d in range(4, DT):
                nc.sync.dma_start(
                    wk_sb[d][:], wk_d.ap()[d * P:(d + 1) * P, :].bitcast(F32R))
            for s in range(4, ST):
                nc.sync.dma_start(x_nats[s][:], x_d.ap()[s * P:(s + 1) * P, :])
            for d in range(DT):
                nc.sync.dma_start(
                    wv_sb[d][:], wv_d.ap()[d * P:(d + 1) * P, :].bitcast(F32R))
            for d in range(DT):
                nc.sync.dma_start(
                    wq_sb[d][:], wq_d.ap()[d * P:(d + 1) * P, :].bitcast(F32R))

            # ---- x transposes (PE) as tiles arrive ----
            for s in range(ST):
                x_nat = x_nats[s]
                for d in range(DT):
                    pt = ps1.tile([P, P], F32, name=f"pt{s}_{d}", tag="pt",
                                  bufs=2)
                    nc.tensor.transpose(pt[:], x_nat[:, d * P:(d + 1) * P],
                                        ident[:])
                    nc.vector.tensor_copy(xT[d][:, s * P:(s + 1) * P], pt[:])

            # ---- k^T projection, c-half outer -> SBUF -> DRAM -> gather.
            # All projection PSUM tiles are [P, 512] halves (tag "proj",
            # 1 bank each): the c=0 half of every e runs before any c=1
            # work, so matmuls start as soon as x s0..3 + wk are in. ----
            for c in range(2):
                for e in range(DT):
                    pk = ps1.tile([P, 512], F32, name=f"pk{c}_{e}",
                                  tag="proj", bufs=4)
                    for d in range(DT):
                        nc.tensor.matmul(
                            pk[:],
                            wk_sb[d][:, e * P:(e + 1) * P],
                            xT[d][:, c * 512:(c + 1) * 512],
                            start=(d == 0), stop=(d == DT - 1))
                    nc.vector.tensor_copy(
                        kT_own[e][:, c * 512:(c + 1) * 512], pk[:])
                    if c == 1:
                        # hi/lo split for the exchange: hi = bf16(kT),
                        # lo = fp8e4m3(kT - hi) (no scale; subnormal flush
                        # only loses ~0.0005-logit precision).
                        # hi-cast on the idle scalar engine (mul by 1.0);
                        # the subtract needs two inputs so it rides vector.
                        # (gpsimd's software DVE is ~4x slower - 3.5us/cast
                        # - and would delay the gather staging by ~30us.)
                        hi = sb.tile([P, D], BF16, name=f"hi{e}", tag="hl",
                                     bufs=2)
                        nc.scalar.mul(hi[:], kT_own[e][:], 1.0)
                        lo = sb.tile([P, D], FP8, name=f"lo{e}", tag="hlo",
                                     bufs=2)
                        nc.vector.tensor_tensor(lo[:], kT_own[e][:], hi[:],
                                                mybir.AluOpType.subtract)
                        nc.scalar.dma_start(
                            send_hi.ap()[e * 64:(e + 1) * 64, :]
                            .bitcast(BF16), hi[:])
                        nc.scalar.dma_start(
                            send_lo.ap()[e * 32:(e + 1) * 32, :]
                            .bitcast(FP8), lo[:])
                        if e == 7:
                            for snd, alc in ((send_hi, allc_hi),
                                             (send_lo, allc_lo)):
                                nc.gpsimd.collective_compute(
                                    "AllGather", mybir.AluOpType.bypass,
                                    replica_groups=REPLICA_GROUPS,
                                    ins=[snd.ap().opt()],
                                    outs=[alc.ap().opt()],
                                )

            # ---- v projection -> own SBUF (bf16) -> send_c (bitcast to
            # f32r: 128 bf16 rows pack into 64 f32r rows) -> AllGather ----
            for s in range(ST):
                for c in range(2):
                    pv = ps1.tile([P, 512], F32, name=f"pv{c}_{s}",
                                  tag="proj", bufs=4)
                    for d in range(DT):
                        nc.tensor.matmul(
                            pv[:],
                            xT[d][:, s * P:(s + 1) * P],
                            wv_sb[d][:, c * 512:(c + 1) * 512],
                            start=(d == 0), stop=(d == DT - 1))
                    nc.vector.tensor_copy(
                        v_own[s][:, c * 512:(c + 1) * 512], pv[:])
                nc.scalar.dma_start(
                    send_c.ap()[s * 64:(s + 1) * 64, :].bitcast(BF16),
                    v_own[s][:])
            nc.gpsimd.collective_compute(
                "AllGather", mybir.AluOpType.bypass,
                replica_groups=REPLICA_GROUPS,
                ins=[send_c.ap().opt()],
                outs=[allc_c.ap().opt()],
            )

            # ---- q^T projection ----
            qT = []
            for e in range(DT):
                qt = sb.tile([P, S_OWN], F32R, name=f"qT{e}", tag="wk0",
                             bufs=8)
                for c in range(2):
                    pq = ps1.tile([P, 512], F32, name=f"pq{c}_{e}",
                                  tag="proj", bufs=4)
                    for d in range(DT):
                        nc.tensor.matmul(
                            pq[:],
                            wq_sb[d][:, e * P:(e + 1) * P],
                            xT[d][:, c * 512:(c + 1) * 512],
                            start=(d == 0), stop=(d == DT - 1))
                    nc.vector.tensor_copy(
                        qt[:, c * 512:(c + 1) * 512], pq[:])
                qT.append(qt)

        # ---- load only the PEER half of the gathers (dynamic row), and
        # reconstruct kT_peer = hi + lo on GPSIMD (idle during the passes;
        # the vector queue would block pass1's softmax work behind these
        # collective-gated adds) ----
        kT_peer = []
        for e in range(DT):
            hi = sb.tile([P, D], BF16, name=f"hip{e}", tag="hl", bufs=2)
            nc.sync.dma_start(
                hi[:],
                allc_hi.ap()[bass.ds(peer, 1), e * 64:(e + 1) * 64, :]
                .bitcast(BF16))
            lo = sb.tile([P, D], FP8, name=f"lop{e}", tag="hlo", bufs=2)
            nc.sync.dma_start(
                lo[:],
                allc_lo.ap()[bass.ds(peer, 1), e * 32:(e + 1) * 32, :]
                .bitcast(FP8))
            t = sb.tile([P, S_OWN], F32R, name=f"kTp{e}", tag="wv0",
                        bufs=8)
            nc.gpsimd.tensor_tensor(t[:], hi[:], lo[:],
                                    mybir.AluOpType.add)
            kT_peer.append(t)
        v_peer = []
        for s in range(ST):
            t = sb.tile([P, D], BF16, name=f"vp{s}", tag="wq0",
                        bufs=8)
            v_peer.append(t)
            nc.sync.dma_start(
                t[:],
                allc_c.ap()[bass.ds(peer, 1),
                            s * 64:(s + 1) * 64, :].bitcast(BF16))

        # ---- attention: flash-style two passes over the key halves.
        # Pass 1 (OWN keys) needs no peer data at all, so it starts
        # right after q-proj (~115us) and fills the window where the
        # baseline stalled waiting for the peer's k^T (~147us). Pass 2
        # (peer keys) starts ~40us after the gather lands -- huge skew
        # margin. Standard flash rescaling makes the result exact:
        #   pass1: m1, l1, O1 = softmax-partial over own keys
        #   pass2: m = max(m1,m2); a = exp((m1-m)/32)
        #          out = (O1*a + O2) / (l1*a + l2)
        kT_half = [kT_own, kT_peer]
        v_half = [v_own, v_peer]
        m1s, l1s, o1s = {}, {}, {}

        with tc.tile_pool(name="ps2", bufs=1, space="PSUM") as ps2:
            state = {}

            def emit_scores(p, sq):
                S_ps = ps2.tile([P, S_OWN], F32, name=f"S{p}_{sq}", tag="S",
                                bufs=2)
                for e in range(DT):
                    for c in range(2):
                        nc.tensor.matmul(
                            S_ps[:, c * 512:(c + 1) * 512],
                            qT[e][:, sq * P:(sq + 1) * P],
                            kT_half[p][e][:, c * 512:(c + 1) * 512],
                            start=(e == 0), stop=(e == DT - 1))
                state[(p, sq)] = S_ps

            def emit_sm1(sq):
                S_ps = state.pop((0, sq))
                m1 = sb.tile([P, 1], F32, name=f"m1_{sq}", tag="m1", bufs=8)
                nc.vector.reduce_max(m1[:], S_ps[:],
                                     axis=mybir.AxisListType.X)
                negm = sb.tile([P, 1], F32, name=f"negm1_{sq}", tag="negm",
                               bufs=2)
                nc.scalar.mul(negm[:], m1[:], -1.0 / 32.0)
                attn = sb.tile([P, S_OWN], BF16, name=f"attn1_{sq}",
                               tag="xa", bufs=3)
                l1 = sb.tile([P, 1], F32, name=f"l1_{sq}", tag="l1", bufs=8)
                nc.scalar.activation(
                    attn[:], S_ps[:], mybir.ActivationFunctionType.Exp,
                    bias=negm[:, 0:1], scale=1.0 / 32.0, accum_out=l1[:])
                m1s[sq], l1s[sq] = m1, l1
                state[(0, sq, "a")] = attn

            def emit_sm2(sq):
                S_ps = state.pop((1, sq))
                m2 = sb.tile([P, 1], F32, name=f"m2_{sq}", tag="m2", bufs=2)
                nc.vector.reduce_max(m2[:], S_ps[:],
                                     axis=mybir.AxisListType.X)
                mm = sb.tile([P, 1], F32, name=f"mm_{sq}", tag="mm", bufs=2)
                nc.vector.tensor_tensor(mm[:], m1s[sq][:], m2[:],
                                        mybir.AluOpType.max)
                negm = sb.tile([P, 1], F32, name=f"negm2_{sq}", tag="negm",
                               bufs=2)
                nc.scalar.mul(negm[:], mm[:], -1.0 / 32.0)
                attn = sb.tile([P, S_OWN], BF16, name=f"attn2_{sq}",
                               tag="xa", bufs=3)
                l2 = sb.tile([P, 1], F32, name=f"l2_{sq}", tag="l2", bufs=2)
                nc.scalar.activation(
                    attn[:], S_ps[:], mybir.ActivationFunctionType.Exp,
                    bias=negm[:, 0:1], scale=1.0 / 32.0, accum_out=l2[:])
                # a = exp((m1 - m)/32); l = l1*a + l2; rl = 1/l
                d1 = sb.tile([P, 1], F32, name=f"d1_{sq}", tag="d1", bufs=2)
                nc.vector.tensor_tensor(d1[:], m1s[sq][:], mm[:],
                                        mybir.AluOpType.subtract)
                alpha = sb.tile([P, 1], F32, name=f"al_{sq}", tag="al",
                                bufs=4)
                nc.scalar.activation(alpha[:], d1[:],
                                     mybir.ActivationFunctionType.Exp,
                                     scale=1.0 / 32.0)
                la = sb.tile([P, 1], F32, name=f"la_{sq}", tag="la", bufs=2)
                nc.vector.tensor_tensor(la[:], l1s[sq][:], alpha[:],
                                        mybir.AluOpType.mult)
                lt = sb.tile([P, 1], F32, name=f"lt_{sq}", tag="lt", bufs=2)
                nc.vector.tensor_tensor(lt[:], la[:], l2[:],
                                        mybir.AluOpType.add)
                rl = sb.tile([P, 1], F32, name=f"rl_{sq}", tag="rl", bufs=4)
                nc.vector.reciprocal(rl[:], lt[:])
                state[(1, sq, "a")] = attn
                state[(sq, "fin")] = (alpha, rl)

            def emit_transp(p, sq):
                # PE block transposes (the XBAR DMA transpose measured only
                # ~45GB/s and collapses under collective DMA contention).
                # All 8 blocks go into ONE psum tile, then ONE vector copy:
                # back-to-back transposes pipeline on the PE instead of
                # round-tripping through per-block copy dependencies.
                attn = state.pop((p, sq, "a"))
                attnT = sb.tile([P, S_OWN], BF16, name=f"aT{p}_{sq}",
                                tag="attnT", bufs=2)
                patb = ps2.tile([P, S_OWN], BF16, name=f"pat{p}_{sq}",
                                tag="pat", bufs=1)
                for t in range(ST):
                    nc.tensor.transpose(
                        patb[:, t * P:(t + 1) * P],
                        attn[:, t * P:(t + 1) * P], identb[:])
                nc.vector.tensor_copy(attnT[:], patb[:])
                state[(p, sq, "T")] = attnT

            def emit_av1(sq):
                attnT = state.pop((0, sq, "T"))
                O_ps = ps2.tile([P, D], F32, name=f"O1_{sq}", tag="O",
                                bufs=1)
                for s in range(ST):
                    for c in range(2):
                        nc.tensor.matmul(
                            O_ps[:, c * 512:(c + 1) * 512],
                            attnT[:, s * P:(s + 1) * P],
                            v_own[s][:, c * 512:(c + 1) * 512],
                            start=(s == 0), stop=(s == ST - 1))
                o1 = sb.tile([P, D], BF16, name=f"o1_{sq}", tag="xT0",
                             bufs=8)
                nc.vector.tensor_copy(o1[:], O_ps[:])
                o1s[sq] = o1

            def emit_av2(sq):
                attnT = state.pop((1, sq, "T"))
                alpha, rl = state.pop((sq, "fin"))
                # sq=6 borrows the (now dead) "S" psum tag so the two drain
                # av2's don't serialize on the single "O" buffer's WAR.
                O_ps = ps2.tile([P, D], F32, name=f"O2_{sq}",
                                tag="S" if sq == 6 else "O",
                                bufs=2 if sq == 6 else 1)
                # c-half OUTER: the h=0 column half finishes accumulating
                # after 8 matmuls, so its combine + out-DMA overlap the
                # h=1 matmuls -- shortens the post-final-matmul tail.
                # out = (o1*alpha + O2) * rl, in [P,512] halves (the halved
                # o_stage tag also frees 4KB/partition of SBUF).
                for h in range(2):
                    hs = slice(h * 512, (h + 1) * 512)
                    for s in range(ST):
                        nc.tensor.matmul(
                            O_ps[:, hs],
                            attnT[:, s * P:(s + 1) * P],
                            v_peer[s][:, hs],
                            start=(s == 0), stop=(s == ST - 1))
                    o_stage = sb.tile([P, 512], F32, name=f"ost{sq}_{h}",
                                      tag="stage", bufs=2)
                    nc.vector.scalar_tensor_tensor(
                        o_stage[:], o1s[sq][:, hs], alpha[:, 0:1],
                        O_ps[:, hs],
                        op0=mybir.AluOpType.mult, op1=mybir.AluOpType.add)
                    nc.vector.tensor_scalar_mul(o_stage[:], o_stage[:],
                                                rl[:, 0:1])
                    nc.scalar.dma_start(
                        out_d.ap()[sq * P:(sq + 1) * P, hs], o_stage[:])

            # pass1 without its drain; pass2 with pass1's last two av's
            # folded into its warmup, so av1(6)/av1(7) have pass2 scores
            # between them instead of stalling back-to-back on the O-psum
            # copy (O bufs=1).
            for sq in range(ST):
                emit_scores(0, sq)
                if sq >= 2:
                    emit_av1(sq - 2)
                emit_sm1(sq)
                emit_transp(0, sq)
            for sq in range(ST + 2):
                if sq < ST:
                    emit_scores(1, sq)
                if sq < 2:
                    emit_av1(ST - 2 + sq)
                else:
                    emit_av2(sq - 2)
                if sq < ST:
                    emit_sm2(sq)
                    emit_transp(1, sq)


_NC_CACHE = {}


def _get_nc():
    if "nc" not in _NC_CACHE:
        _NC_CACHE["nc"] = build_kernel()
    return _NC_CACHE["nc"]


def kernel(x, Wq, Wk, Wv, **_ignored):
    x = np.ascontiguousarray(np.asarray(x, dtype=np.float32))
    Wq = np.ascontiguousarray(np.asarray(Wq, dtype=np.float32))
    Wk = np.ascontiguousarray(np.asarray(Wk, dtype=np.float32))
    Wv = np.ascontiguousarray(np.asarray(Wv, dtype=np.float32))
    nc = _get_nc()
    in_maps = []
    for c in range(NCORES):
        b, h = divmod(c, 2)
        in_maps.append({
            "x": x[b, h * S_OWN:(h + 1) * S_OWN, :],
            "Wq": Wq, "Wk": Wk, "Wv": Wv,
        })
    res = run_bass_kernel_spmd(nc, in_maps, core_ids=list(range(NCORES)))
    out = np.empty((B, S_FULL, D), dtype=np.float32)
    for c in range(NCORES):
        b, h = divmod(c, 2)
        out[b, h * S_OWN:(h + 1) * S_OWN, :] = res.results[c]["out"]
    return out



# revision 51
# speedup vs baseline: 1.1127x; 1.0010x over previous
"""Trainium2 Bass kernel for single-head attention.

Reference computation (per batch b):
    q = x @ Wq; k = x @ Wk; v = x @ Wv          # x: [S, D], W: [D, D]
    out = softmax(q @ k.T / sqrt(D)) @ v

Shapes: B=4, S=2048, D=1024, f32.

Sharding over 8 NeuronCores: core c -> (batch b = c//2, seq half h = c%2).
Each core:
  - computes q^T, k^T (layout [e, s]) and v ([s, e]) for its own S/2 rows
  - AllGathers k^T (bf16 hi + fp8 lo residual) and v (bf16) within the
    pair {2b, 2b+1}
  - computes scores for its 1024 queries vs all 2048 keys, softmax,
    attn @ v, writes its [1024, 1024] output shard.

dtype strategy (validated empirically):
  - all matmuls in float32r (~13-bit mantissa; end-to-end rel err ~9e-3
    vs the f32 reference, under the 2e-2 gate)
  - attn weights / gathered v in bf16 (error enters output linearly).

Scheduling (v15, ~253us; v6 was ~269us, session baseline 301us):
  - own k^T/v stay resident in SBUF after projection; only the PEER
    half is loaded from the gather output, via a dynamic-offset DMA
    (row index = 1 - partition_id%2).
  - input loads ride the sync queue in priority order ({x s0..3 + wk
    interleaved}, wk4..7, x4..7, wv, wq); load DMA bandwidth is
    ring-shared ~230GB/s, so multi-queue splits don't help. Staging
    rides scalar; collective triggers ride gpsimd; an inline-tensor
    t=0 barrier absorbs the CC engine's ~30us first-collective arming.
  - projections run with [P,512] half-tile PSUM accumulators, k-proj
    c-half-OUTER so matmuls start once {x s0..3, wk} (6.3MB) are in.
  - the 6MB pair exchange is compressed to 5MB in three <=2MB
    ALGO_MESH AllGathers (>2MB falls into ~5x-slower ALGO_RING): kT as
    bf16 hi + fp8e4m3 residual (reconstructed hi+lo on gpsimd, which
    idles during the passes), v as bf16 bitcast into f32r rows.
  - attention is FLASH-STYLE TWO-PASS over the key halves: pass 1
    (own keys: scores -> partial softmax with own max m1/l1 ->
    transpose -> attn@v into an unnormalized bf16 partial) needs no
    peer data and starts right after q-proj (~115us), filling the
    window where a single-pass kernel stalls waiting for the peer's
    k^T (~147us, pair-launch skew ~48us). Pass 2 (peer keys) starts
    ~40us after the gather lands and combines exactly:
      m = max(m1,m2); a = exp((m1-m)/32)
      out = (O1*a + O2) / (l1*a + l2)
    Both passes run the proven PE pipeline with scores two tiles
    ahead of attn@v.
  - pass1's 2-tile drain is folded into pass2's warmup (scores2
    between av1(6)/av1(7)), and av2(6) borrows the then-dead "S" psum
    tag, so the O-psum (bufs=1) never serializes back-to-back av's.
Run-to-run variance: under sustained load the chip drops to the P0
power state (PE 2.38 -> 2.0 GHz); identical NEFFs then measure ~1.2x
slower. 512-col matmul min dur in the trace tells the state: 215ns =
full clock, 256ns = P0. Launch skew between pair cores (~8-20us)
shifts the whole CC chain; the ~10us CC slack absorbs it.
"""

import numpy as np

import concourse.bass as bass
import concourse.mybir as mybir
import concourse.tile as tile
from concourse import bacc
from concourse.bass_utils import run_bass_kernel_spmd

P = 128          # partitions
D = 1024         # model dim (= E)
S_OWN = 1024     # sequence rows per core
S_FULL = 2048
B, NCORES = 4, 8
DT = D // P      # 8 d-tiles
ST = S_OWN // P  # 8 s-tiles
F32 = mybir.dt.float32
F32R = mybir.dt.float32r
BF16 = mybir.dt.bfloat16
FP8 = mybir.dt.float8e4
REPLICA_GROUPS = [[0, 1], [2, 3], [4, 5], [6, 7]]


def build_kernel():
    nc = bacc.Bacc("TRN2", target_bir_lowering=False, num_devices=NCORES)

    x_d = nc.dram_tensor("x", [S_OWN, D], F32, kind="ExternalInput")
    wq_d = nc.dram_tensor("Wq", [D, D], F32, kind="ExternalInput")
    wk_d = nc.dram_tensor("Wk", [D, D], F32, kind="ExternalInput")
    wv_d = nc.dram_tensor("Wv", [D, D], F32, kind="ExternalInput")
    out_d = nc.dram_tensor("out", [S_OWN, D], F32, kind="ExternalOutput")

    # Collective bounce buffers (internal DRAM). Anything over 2MB switches
    # NRT from ALGO_MESH to the ~4-5x slower ALGO_RING (measured: 4MB kT
    # gather 109us, 3MB chunks 81-86us, vs ~25-40us for <=2MB mesh ops),
    # and each mesh op costs ~6us fixed + ~2us gap on the serialized CC
    # engine. So the 6MB exchange is compressed to 5MB in three mesh ops:
    # kT as bf16 hi (2MB) + fp8e4m3 residual lo (1MB) -- numerically ~free,
    # emulated rel err 0.0042 vs 0.0029 for full f32r -- plus v (2MB bf16).
    # All gathers are bitcast into f32r row-tensors.
    send_hi = nc.dram_tensor("send_hi", [4 * P, S_OWN], F32R)
    allc_hi = nc.dram_tensor("allc_hi", [2, 4 * P, S_OWN], F32R)
    send_lo = nc.dram_tensor("send_lo", [2 * P, S_OWN], F32R)
    allc_lo = nc.dram_tensor("allc_lo", [2, 2 * P, S_OWN], F32R)
    send_c = nc.dram_tensor("send_c", [4 * P, S_OWN], F32R)
    allc_c = nc.dram_tensor("allc_c", [2, 4 * P, S_OWN], F32R)

    # bar_send is an inline (NEFF-preloaded) tensor so the t=0 barrier
    # collective has NO producer dependency and triggers immediately; its
    # ~34us CC arming then completes by ~45us instead of ~65us, pulling the
    # whole serialized CC chain (bar, kt0, kt1, v0, v1) ~20us earlier.
    bar_send = nc.inline_tensor(np.zeros((1, 128), np.float32),
                                name="bar_send")
    bar_out = nc.dram_tensor("bar_out", [2, 128], F32)

    ident_np = np.eye(P, dtype=np.float32)
    ident_d = nc.inline_tensor(ident_np, name="ident")

    with tile.TileContext(nc) as tc:
        _emit(nc, tc, x_d, wq_d, wk_d, wv_d, out_d,
              send_hi, allc_hi, send_lo, allc_lo, send_c, allc_c,
              ident_d, bar_send, bar_out)
    nc.compile()
    return nc


def _emit(nc, tc, x_d, wq_d, wk_d, wv_d, out_d,
          send_hi, allc_hi, send_lo, allc_lo, send_c, allc_c,
          ident_d, bar_send, bar_out):
    with tc.tile_pool(name="sb", bufs=1) as sb:
        ident = sb.tile([P, P], F32, name="ident")
        nc.sync.dma_start(ident[:], ident_d.ap())
        identb = sb.tile([P, P], BF16, name="identb")
        nc.gpsimd.dma_start(identb[:], ident_d.ap())  # cast f32->bf16

        # tiny AllGather at t=0: pays the CC engine's ~35-40us
        # first-collective arming latency during the load phase, so the
        # kT gather processes immediately when its data is staged
        nc.gpsimd.collective_compute(
            "AllGather", mybir.AluOpType.bypass,
            replica_groups=REPLICA_GROUPS,
            ins=[bar_send.ap().opt()],
            outs=[bar_out.ap().opt()],
        )

        # which gather-output row is the peer's (0 or 1)
        peer = 1 - (nc.sync.partition_id() % 2)

        # SBUF tag plan (KB/partition, 207.9 usable). Generational reuse:
        #   wk0: wk(8x4K)  -> qT(8x4K)        [wk dies at kT-proj end]
        #   wv0: wv(8x4K)  -> kT_peer(8x4K)   [wv dies at v-proj end]
        #   wq0: wq(8x4K)  -> v_peer(8x2K)    [wq dies at q-proj end]
        #   xT0: xT(8x4K)                     [dies at q-proj end]
        #   kTo: own k^T, 8x4K dedicated
        #   vo:  own v, 8x2K dedicated
        #   xa:  x_nat(3 bufs) -> attn(3 bufs); attnT 2 bufs; stage 2 bufs
        wk_sb = [sb.tile([P, D], F32R, name=f"wk{d}", tag="wk0", bufs=8)
                 for d in range(DT)]
        wv_sb = [sb.tile([P, D], F32R, name=f"wv{d}", tag="wv0", bufs=8)
                 for d in range(DT)]
        wq_sb = [sb.tile([P, D], F32R, name=f"wq{d}", tag="wq0", bufs=8)
                 for d in range(DT)]
        xT = [sb.tile([P, S_OWN], F32R, name=f"xT{d}", tag="xT0", bufs=8)
              for d in range(DT)]
        kT_own = [sb.tile([P, S_OWN], F32R, name=f"kTo{e}", tag="kTo",
                          bufs=8) for e in range(DT)]
        v_own = [sb.tile([P, D], BF16, name=f"vo{s}", tag="vo", bufs=8)
                 for s in range(ST)]

        with tc.tile_pool(name="ps1", bufs=1, space="PSUM") as ps1:
            # ---- input loads: one queue, priority order x/wk, wv, wq.
            # (Measured: load DMA bandwidth is ring-shared ~230GB/s, so
            # splitting loads across sync+scalar queues buys nothing; keep
            # them all on sync so scalar is free for kT staging.) ----
            # Load order: [x0..x3 + wk0..wk3 interleaved, wk4..7, x4..7,
            # wv, wq]. The k projection runs c-half-OUTER below, and its
            # c=0 half needs exactly {x s0..3, all wk} = the first 6.3MB of
            # this stream, so PE projection work starts at ~37us instead of
            # waiting for all of x+wk (~46us).
            x_nats = []
            for s in range(ST):
                x_nat = sb.tile([P, D], F32, name=f"x_nat{s}", tag="xa",
                                bufs=3)
                x_nats.append(x_nat)
            for i in range(4):
                nc.sync.dma_start(x_nats[i][:], x_d.ap()[i * P:(i + 1) * P, :])
                nc.sync.dma_start(
                    wk_sb[i][:], wk_d.ap()[i * P:(i + 1) * P, :].bitcast(F32R))
            for d in range(4, DT):
                nc.sync.dma_start(
                    wk_sb[d][:], wk_d.ap()[d * P:(d + 1) * P, :].bitcast(F32R))
            for s in range(4, ST):
                nc.sync.dma_start(x_nats[s][:], x_d.ap()[s * P:(s + 1) * P, :])
            for d in range(DT):
                nc.sync.dma_start(
                    wv_sb[d][:], wv_d.ap()[d * P:(d + 1) * P, :].bitcast(F32R))
            for d in range(DT):
                nc.sync.dma_start(
                    wq_sb[d][:], wq_d.ap()[d * P:(d + 1) * P, :].bitcast(F32R))

            # ---- x transposes (PE) as tiles arrive ----
            for s in range(ST):
                x_nat = x_nats[s]
                for d in range(DT):
                    pt = ps1.tile([P, P], F32, name=f"pt{s}_{d}", tag="pt",
                                  bufs=2)
                    nc.tensor.transpose(pt[:], x_nat[:, d * P:(d + 1) * P],
                                        ident[:])
                    nc.vector.tensor_copy(xT[d][:, s * P:(s + 1) * P], pt[:])

            # ---- k^T projection, c-half outer -> SBUF -> DRAM -> gather.
            # All projection PSUM tiles are [P, 512] halves (tag "proj",
            # 1 bank each): the c=0 half of every e runs before any c=1
            # work, so matmuls start as soon as x s0..3 + wk are in. ----
            for c in range(2):
                for e in range(DT):
                    pk = ps1.tile([P, 512], F32, name=f"pk{c}_{e}",
                                  tag="proj", bufs=4)
                    for d in range(DT):
                        nc.tensor.matmul(
                            pk[:],
                            wk_sb[d][:, e * P:(e + 1) * P],
                            xT[d][:, c * 512:(c + 1) * 512],
                            start=(d == 0), stop=(d == DT - 1))
                    nc.vector.tensor_copy(
                        kT_own[e][:, c * 512:(c + 1) * 512], pk[:])
                    if c == 1:
                        # hi/lo split for the exchange: hi = bf16(kT),
                        # lo = fp8e4m3(kT - hi) (no scale; subnormal flush
                        # only loses ~0.0005-logit precision).
                        # hi-cast on the idle scalar engine (mul by 1.0);
                        # the subtract needs two inputs so it rides vector.
                        # (gpsimd's software DVE is ~4x slower - 3.5us/cast
                        # - and would delay the gather staging by ~30us.)
                        hi = sb.tile([P, D], BF16, name=f"hi{e}", tag="hl",
                                     bufs=2)
                        nc.scalar.mul(hi[:], kT_own[e][:], 1.0)
                        lo = sb.tile([P, D], FP8, name=f"lo{e}", tag="hlo",
                                     bufs=2)
                        nc.vector.tensor_tensor(lo[:], kT_own[e][:], hi[:],
                                                mybir.AluOpType.subtract)
                        nc.scalar.dma_start(
                            send_hi.ap()[e * 64:(e + 1) * 64, :]
                            .bitcast(BF16), hi[:])
                        nc.scalar.dma_start(
                            send_lo.ap()[e * 32:(e + 1) * 32, :]
                            .bitcast(FP8), lo[:])
                        if e == 7:
                            for snd, alc in ((send_hi, allc_hi),
                                             (send_lo, allc_lo)):
                                nc.gpsimd.collective_compute(
                                    "AllGather", mybir.AluOpType.bypass,
                                    replica_groups=REPLICA_GROUPS,
                                    ins=[snd.ap().opt()],
                                    outs=[alc.ap().opt()],
                                )

            # ---- v projection -> own SBUF (bf16) -> send_c (bitcast to
            # f32r: 128 bf16 rows pack into 64 f32r rows) -> AllGather ----
            for s in range(ST):
                for c in range(2):
                    pv = ps1.tile([P, 512], F32, name=f"pv{c}_{s}",
                                  tag="proj", bufs=4)
                    for d in range(DT):
                        nc.tensor.matmul(
                            pv[:],
                            xT[d][:, s * P:(s + 1) * P],
                            wv_sb[d][:, c * 512:(c + 1) * 512],
                            start=(d == 0), stop=(d == DT - 1))
                    nc.vector.tensor_copy(
                        v_own[s][:, c * 512:(c + 1) * 512], pv[:])
                nc.scalar.dma_start(
                    send_c.ap()[s * 64:(s + 1) * 64, :].bitcast(BF16),
                    v_own[s][:])
            nc.gpsimd.collective_compute(
                "AllGather", mybir.AluOpType.bypass,
                replica_groups=REPLICA_GROUPS,
                ins=[send_c.ap().opt()],
                outs=[allc_c.ap().opt()],
            )

            # ---- q^T projection ----
            qT = []
            for e in range(DT):
                qt = sb.tile([P, S_OWN], F32R, name=f"qT{e}", tag="wk0",
                             bufs=8)
                for c in range(2):
                    pq = ps1.tile([P, 512], F32, name=f"pq{c}_{e}",
                                  tag="proj", bufs=4)
                    for d in range(DT):
                        nc.tensor.matmul(
                            pq[:],
                            wq_sb[d][:, e * P:(e + 1) * P],
                            xT[d][:, c * 512:(c + 1) * 512],
                            start=(d == 0), stop=(d == DT - 1))
                    nc.vector.tensor_copy(
                        qt[:, c * 512:(c + 1) * 512], pq[:])
                qT.append(qt)

        # ---- load only the PEER half of the gathers (dynamic row), and
        # reconstruct kT_peer = hi + lo on GPSIMD (idle during the passes;
        # the vector queue would block pass1's softmax work behind these
        # collective-gated adds) ----
        kT_peer = []
        for e in range(DT):
            hi = sb.tile([P, D], BF16, name=f"hip{e}", tag="hl", bufs=2)
            nc.sync.dma_start(
                hi[:],
                allc_hi.ap()[bass.ds(peer, 1), e * 64:(e + 1) * 64, :]
                .bitcast(BF16))
            lo = sb.tile([P, D], FP8, name=f"lop{e}", tag="hlo", bufs=2)
            nc.sync.dma_start(
                lo[:],
                allc_lo.ap()[bass.ds(peer, 1), e * 32:(e + 1) * 32, :]
                .bitcast(FP8))
            t = sb.tile([P, S_OWN], F32R, name=f"kTp{e}", tag="wv0",
                        bufs=8)
            nc.gpsimd.tensor_tensor(t[:], hi[:], lo[:],
                                    mybir.AluOpType.add)
            kT_peer.append(t)
        v_peer = []
        for s in range(ST):
            t = sb.tile([P, D], BF16, name=f"vp{s}", tag="wq0",
                        bufs=8)
            v_peer.append(t)
            nc.sync.dma_start(
                t[:],
                allc_c.ap()[bass.ds(peer, 1),
                            s * 64:(s + 1) * 64, :].bitcast(BF16))

        # ---- attention: flash-style two passes over the key halves.
        # Pass 1 (OWN keys) needs no peer data at all, so it starts
        # right after q-proj (~115us) and fills the window where the
        # baseline stalled waiting for the peer's k^T (~147us). Pass 2
        # (peer keys) starts ~40us after the gather lands -- huge skew
        # margin. Standard flash rescaling makes the result exact:
        #   pass1: m1, l1, O1 = softmax-partial over own keys
        #   pass2: m = max(m1,m2); a = exp((m1-m)/32)
        #          out = (O1*a + O2) / (l1*a + l2)
        kT_half = [kT_own, kT_peer]
        v_half = [v_own, v_peer]
        m1s, l1s, o1s = {}, {}, {}

        with tc.tile_pool(name="ps2", bufs=1, space="PSUM") as ps2:
            state = {}

            def emit_scores(p, sq):
                S_ps = ps2.tile([P, S_OWN], F32, name=f"S{p}_{sq}", tag="S",
                                bufs=2)
                for e in range(DT):
                    for c in range(2):
                        nc.tensor.matmul(
                            S_ps[:, c * 512:(c + 1) * 512],
                            qT[e][:, sq * P:(sq + 1) * P],
                            kT_half[p][e][:, c * 512:(c + 1) * 512],
                            start=(e == 0), stop=(e == DT - 1))
                state[(p, sq)] = S_ps

            def emit_sm1(sq):
                S_ps = state.pop((0, sq))
                m1 = sb.tile([P, 1], F32, name=f"m1_{sq}", tag="m1", bufs=8)
                nc.vector.reduce_max(m1[:], S_ps[:],
                                     axis=mybir.AxisListType.X)
                negm = sb.tile([P, 1], F32, name=f"negm1_{sq}", tag="negm",
                               bufs=2)
                nc.scalar.mul(negm[:], m1[:], -1.0 / 32.0)
                attn = sb.tile([P, S_OWN], BF16, name=f"attn1_{sq}",
                               tag="xa", bufs=3)
                l1 = sb.tile([P, 1], F32, name=f"l1_{sq}", tag="l1", bufs=8)
                nc.scalar.activation(
                    attn[:], S_ps[:], mybir.ActivationFunctionType.Exp,
                    bias=negm[:, 0:1], scale=1.0 / 32.0, accum_out=l1[:])
                m1s[sq], l1s[sq] = m1, l1
                state[(0, sq, "a")] = attn

            def emit_sm2(sq):
                S_ps = state.pop((1, sq))
                m2 = sb.tile([P, 1], F32, name=f"m2_{sq}", tag="m2", bufs=2)
                nc.vector.reduce_max(m2[:], S_ps[:],
                                     axis=mybir.AxisListType.X)
                mm = sb.tile([P, 1], F32, name=f"mm_{sq}", tag="mm", bufs=2)
                nc.vector.tensor_tensor(mm[:], m1s[sq][:], m2[:],
                                        mybir.AluOpType.max)
                negm = sb.tile([P, 1], F32, name=f"negm2_{sq}", tag="negm",
                               bufs=2)
                nc.scalar.mul(negm[:], mm[:], -1.0 / 32.0)
                attn = sb.tile([P, S_OWN], BF16, name=f"attn2_{sq}",
                               tag="xa", bufs=3)
                l2 = sb.tile([P, 1], F32, name=f"l2_{sq}", tag="l2", bufs=2)
                nc.scalar.activation(
                    attn[:], S_ps[:], mybir.ActivationFunctionType.Exp,
                    bias=negm[:, 0:1], scale=1.0 / 32.0, accum_out=l2[:])
                # a = exp((m1 - m)/32); l = l1*a + l2; rl = 1/l
                d1 = sb.tile([P, 1], F32, name=f"d1_{sq}", tag="d1", bufs=2)
                nc.vector.tensor_tensor(d1[:], m1s[sq][:], mm[:],
                                        mybir.AluOpType.subtract)
                alpha = sb.tile([P, 1], F32, name=f"al_{sq}", tag="al",
                                bufs=4)
                nc.scalar.activation(alpha[:], d1[:],
                                     mybir.ActivationFunctionType.Exp,
                                     scale=1.0 / 32.0)
                la = sb.tile([P, 1], F32, name=f"la_{sq}", tag="la", bufs=2)
                nc.vector.tensor_tensor(la[:], l1s[sq][:], alpha[:],
                                        mybir.AluOpType.mult)
                lt = sb.tile([P, 1], F32, name=f"lt_{sq}", tag="lt", bufs=2)
                nc.vector.tensor_tensor(lt[:], la[:], l2[:],
                                        mybir.AluOpType.add)
                rl = sb.tile([P, 1], F32, name=f"rl_{sq}", tag="rl", bufs=4)
                nc.vector.reciprocal(rl[:], lt[:])
                state[(1, sq, "a")] = attn
                state[(sq, "fin")] = (alpha, rl)

            def emit_transp(p, sq):
                # PE block transposes (the XBAR DMA transpose measured only
                # ~45GB/s and collapses under collective DMA contention).
                # All 8 blocks go into ONE psum tile, then ONE vector copy:
                # back-to-back transposes pipeline on the PE instead of
                # round-tripping through per-block copy dependencies.
                attn = state.pop((p, sq, "a"))
                attnT = sb.tile([P, S_OWN], BF16, name=f"aT{p}_{sq}",
                                tag="attnT", bufs=2)
                patb = ps2.tile([P, S_OWN], BF16, name=f"pat{p}_{sq}",
                                tag="pat", bufs=1)
                for t in range(ST):
                    nc.tensor.transpose(
                        patb[:, t * P:(t + 1) * P],
                        attn[:, t * P:(t + 1) * P], identb[:])
                nc.vector.tensor_copy(attnT[:], patb[:])
                state[(p, sq, "T")] = attnT

            def emit_av1(sq):
                # column halves in separate 1-bank psum tiles (tags Oh0/
                # Oh1): no write-after-read coupling between the halves'
                # accumulation chains and their copies.
                attnT = state.pop((0, sq, "T"))
                o1 = sb.tile([P, D], BF16, name=f"o1_{sq}", tag="xT0",
                             bufs=8)
                for h in range(2):
                    hs = slice(h * 512, (h + 1) * 512)
                    O_ps = ps2.tile([P, 512], F32, name=f"O1_{sq}_{h}",
                                    tag=f"Oh{h}", bufs=1)
                    for s in range(ST):
                        nc.tensor.matmul(
                            O_ps[:],
                            attnT[:, s * P:(s + 1) * P],
                            v_own[s][:, hs],
                            start=(s == 0), stop=(s == ST - 1))
                    nc.vector.tensor_copy(o1[:, hs], O_ps[:])
                o1s[sq] = o1

            def emit_av2(sq):
                attnT = state.pop((1, sq, "T"))
                alpha, rl = state.pop((sq, "fin"))
                # sq=6 borrows the (now dead) "S" psum tag so the two drain
                # av2's don't serialize on the single "O" buffer's WAR.
                # c-half OUTER in separate 1-bank psum tiles: the h=0
                # half's combine + out-DMA overlap the h=1 matmuls (no WAR
                # coupling), shortening the post-final-matmul tail.
                # out = (o1*alpha + O2) * rl, in [P,512] halves (the halved
                # o_stage tag also frees 4KB/partition of SBUF).
                for h in range(2):
                    hs = slice(h * 512, (h + 1) * 512)
                    O_ps = ps2.tile([P, 512], F32, name=f"O2_{sq}_{h}",
                                    tag=f"Oh{h}", bufs=1)
                    for s in range(ST):
                        nc.tensor.matmul(
                            O_ps[:],
                            attnT[:, s * P:(s + 1) * P],
                            v_peer[s][:, hs],
                            start=(s == 0), stop=(s == ST - 1))
                    o_stage = sb.tile([P, 512], F32, name=f"ost{sq}_{h}",
                                      tag="stage", bufs=2)
                    nc.vector.scalar_tensor_tensor(
                        o_stage[:], o1s[sq][:, hs], alpha[:, 0:1],
                        O_ps[:],
                        op0=mybir.AluOpType.mult, op1=mybir.AluOpType.add)
                    nc.vector.tensor_scalar_mul(o_stage[:], o_stage[:],
                                                rl[:, 0:1])
                    nc.scalar.dma_start(
                        out_d.ap()[sq * P:(sq + 1) * P, hs], o_stage[:])

            # pass1 without its drain; pass2 with pass1's last two av's
            # folded into its warmup, so av1(6)/av1(7) have pass2 scores
            # between them instead of stalling back-to-back on the O-psum
            # copy (O bufs=1).
            for sq in range(ST):
                emit_scores(0, sq)
                if sq >= 2:
                    emit_av1(sq - 2)
                emit_sm1(sq)
                emit_transp(0, sq)
            for sq in range(ST + 2):
                if sq < ST:
                    emit_scores(1, sq)
                if sq < 2:
                    emit_av1(ST - 2 + sq)
                else:
                    emit_av2(sq - 2)
                if sq < ST:
                    emit_sm2(sq)
                    emit_transp(1, sq)


_NC_CACHE = {}


def _get_nc():
    if "nc" not in _NC_CACHE:
        _NC_CACHE["nc"] = build_kernel()
    return _NC_CACHE["nc"]


def kernel(x, Wq, Wk, Wv, **_ignored):
    x = np.ascontiguousarray(np.asarray(x, dtype=np.float32))
    Wq = np.ascontiguousarray(np.asarray(Wq, dtype=np.float32))
    Wk = np.ascontiguousarray(np.asarray(Wk, dtype=np.float32))
    Wv = np.ascontiguousarray(np.asarray(Wv, dtype=np.float32))
    nc = _get_nc()
    in_maps = []
    for c in range(NCORES):
        b, h = divmod(c, 2)
        in_maps.append({
            "x": x[b, h * S_OWN:(h + 1) * S_OWN, :],
            "Wq": Wq, "Wk": Wk, "Wv": Wv,
        })
    res = run_bass_kernel_spmd(nc, in_maps, core_ids=list(range(NCORES)))
    out = np.empty((B, S_FULL, D), dtype=np.float32)
    for c in range(NCORES):
        b, h = divmod(c, 2)
        out[b, h * S_OWN:(h + 1) * S_OWN, :] = res.results[c]["out"]
    return out

